# revision 39
# baseline (speedup 1.0000x reference)
"""Multi-head attention (B=2, S=2048, H=16, D=64) on 8 Trainium2 NeuronCores.

Head-parallel tensor parallelism: core c owns heads {2c, 2c+1} (a 128-dim
slice of the model dim): column-parallel QKV projections and local causal
attention for its 2 heads, then an AllToAll of bf16 context vectors (one
512-token query group at a time) and a full-width Wo projection for this
core's own disjoint 64-token output slices.

Schedule (v6), shaped by trace measurements:

* q/k projections run in fp8e4m3 with perf_mode=DoubleRow (256-wide
  contraction chunks, ~1.5x PE throughput); Wq/Wk are pre-scaled by 64 on
  the host (their 0.02-sigma entries would land in e4m3's subnormal range)
  and the 64*64 logit factor is undone inside the exp's free scale.  The
  V projection stays bf16: early-token v errors are unprotected by
  softmax averaging and would breach the error budget.
* Startup: wq8/bq and the fp8 x tile 0 are the first DMAs issued;
  everything else loads behind them in deadline order (tri before the
  first diagonal block, wo/bo last).
* Query groups run in order {2, 3, 0, 1} per batch so the LAST collective
  (half 0, only 12 key-blocks of attention) fires well before the PE
  drains.  4 collectives spaced >=15 us: per-qt granularity (8
  collectives) was measured to DEGRADE - back-to-back AllToAlls on this
  part grow from ~6 us to ~22 us each - and the first collective
  completes no earlier than ~100 us (first-call barrier), so every
  collective-dependent Wo block is placed with >=20 us of slack and the
  last Wo blocks are held back as PE filler for the final collective.
* PSUM: proj pool 2 banks (q/k/v/transpose rotate through [128,512] slots),
  scores 2x2 banks, ctx accumulators 2 banks = exactly 8.  The projection
  for tile t+1 and the Wo matmuls are emitted after the attention section
  that hides them; the Tile scheduler slots them into the PE stalls where
  attention waits on the ACT exp stream (exp on ACT is ~1.15 us per
  128-key block vs ~0.65 us of PE work, so without filler the PE idles
  ~40% during attention and the HAM clock gate re-throttles).
* Softmax normalization happens on the receiving core (the a2a payload is
  65 rows per head: 64 unnormalized ctx dims + the denominator row from a
  trailing ones-column in the AV stationary); 16 denominator rows stack on
  the partition axis at the receiver: a 16-lane DVE reciprocal, then 8
  K=16 one-hot selector matmuls broadcast the reciprocals into PSUM in cg
  layout (replacing a DRAM-bounced stride-0 DMA round trip, and nudging
  the PE awake right before the Wo block), then one fused DVE multiply
  normalizes the gathered ctx.  Tail recv chains issue their gather DMAs
  from the Scalar queue (idle in the tail; Sync is not).
* Attention-times-V keeps V plus a trailing ones column as the 65-column
  stationary operand and streams the exp tile; scores use tile_position
  row pairs so the two heads' score matmuls run concurrently; exp is one
  ACT instruction per key block covering both heads; the diagonal tri-mask
  is one DVE multiply per block via a stride-0 broadcast AP over heads.
* A tiny warm-up AllToAll is issued during the load phase so the first real
  collective doesn't pay the ~23 us first-call setup on the critical path.
"""

import sys

sys.path.insert(0, "/opt/trn_rl_repo")

import ml_dtypes
import numpy as np

import concourse.bass as bass
import concourse.tile as tile
from concourse import bacc, mybir
from concourse.bass_utils import run_bass_kernel_spmd

N_CORES = 8
B, S, H, D = 2, 2048, 16, 64
E = H * D            # 1024
T = B * S            # 4096 tokens
DPC = 128            # dims (2 heads) per core
NKC = E // 128       # 8 contraction chunks for the projections
SB = S // 128        # 16 key blocks per batch
PHQ = 512 // N_CORES  # 64 tokens per core per query group
PH = 2 * PHQ         # 128 tokens per core per half-batch
CR = 130             # a2a chunk rows: 2 x (64 ctx dims + den)

F32 = mybir.dt.float32
BF16 = mybir.dt.bfloat16
FP8 = mybir.dt.float8e4
AFT = mybir.ActivationFunctionType
NK2 = 4              # 256-wide contraction chunks for DoubleRow q/k

QT_ORDER = (0, 1, 3, 2)  # hf0 = {0,1} finishes early; q2 (12 blocks) last


def build_program():
    nc = bacc.Bacc("TRN2", target_bir_lowering=False, debug=False,
                   num_devices=N_CORES)

    xT = nc.dram_tensor("xT", [E, T], BF16, kind="ExternalInput").ap()
    # fp8 copies of x and Wq/Wk for DoubleRow q/k projections (v stays
    # bf16: early-token v errors are unprotected by softmax averaging)
    x8T = nc.dram_tensor("x8T", [E, T], FP8, kind="ExternalInput").ap()
    wq8T = nc.dram_tensor("wq8T", [E, DPC], FP8, kind="ExternalInput").ap()
    wk8T = nc.dram_tensor("wk8T", [E, DPC], FP8, kind="ExternalInput").ap()
    wvT = nc.dram_tensor("wvT", [E, DPC], BF16, kind="ExternalInput").ap()
    woT = nc.dram_tensor("woT", [E, E], BF16, kind="ExternalInput").ap()
    bq = nc.dram_tensor("bq", [DPC, 1], F32, kind="ExternalInput").ap()
    bk = nc.dram_tensor("bk", [DPC, 1], F32, kind="ExternalInput").ap()
    bv = nc.dram_tensor("bv", [DPC, 1], F32, kind="ExternalInput").ap()
    bo = nc.dram_tensor("bo", [E], F32, kind="ExternalInput").ap()
    # single 128x128 lower-triangular (k_local <= q_local) mask
    tri = nc.dram_tensor("tri", [128, 128], BF16, kind="ExternalInput").ap()
    ident = nc.dram_tensor("ident", [128, 128], BF16, kind="ExternalInput").ap()
    # selk[k, kc*128+p] = 1 iff k == 2*kc + p//64: one-hot selectors that
    # broadcast the 16 per-(peer,head) reciprocal rows to the cg layout
    # via 8 tiny K=16 matmuls (replaces a DRAM-bounced stride-0 DMA).
    selk = nc.dram_tensor("selk", [16, NKC * 128], BF16,
                          kind="ExternalInput").ap()
    out = nc.dram_tensor("out", [T // N_CORES, E], F32, kind="ExternalOutput").ap()

    with tile.TileContext(nc) as tc:
        with (
            tc.tile_pool(name="consts", bufs=1) as consts,
            tc.tile_pool(name="state", bufs=1) as state,
            tc.tile_pool(name="ep", bufs=6) as ep,
            tc.tile_pool(name="op", bufs=4) as op,
            tc.tile_pool(name="ps_p", bufs=2, space="PSUM") as ps_p,
            tc.tile_pool(name="ps_s", bufs=2, space="PSUM") as ps_s,
            tc.tile_pool(name="ps_c", bufs=2, space="PSUM") as ps_c,
            tc.tile_pool(name="dram", bufs=1, space="DRAM") as dram,
        ):
            # ---- warm-up collective: absorbs the first-AllToAll setup cost
            # while the DMA engines are still loading x ----------------------
            wu_s = consts.tile([128, 16], BF16)
            nc.vector.memset(wu_s[:], 0.0)
            wu_in = dram.tile([N_CORES, 16, 16], BF16, tag="wu_in", name="wu_in")
            wu_out = dram.tile([N_CORES, 16, 16], BF16, tag="wu_out",
                               name="wu_out")
            nc.sync.dma_start(out=wu_in[:], in_=wu_s[:])
            nc.gpsimd.collective_compute(
                "AllToAll",
                mybir.AluOpType.bypass,
                replica_groups=[list(range(N_CORES))],
                ins=[wu_in.opt()],
                outs=[wu_out.opt()],
            )

            def chunked(dram_ap, cols, kc0, kcn):
                # DRAM [E, cols] viewed as [p, kc, cols]: row kc*128+p
                return bass.AP(tensor=dram_ap.tensor,
                               offset=dram_ap.offset + kc0 * 128 * cols,
                               ap=[[cols, 128], [128 * cols, kcn], [1, cols]])

            # ---- loads in deadline order ---------------------------------
            # fp8 weights/x in DoubleRow layout: contraction index
            # k = kc2*256 + i*128 + p -> [p, kc2, i, cols]
            def dr_ap(dram_ap, cols, toff=0, width=None):
                return bass.AP(tensor=dram_ap.tensor,
                               offset=dram_ap.offset + toff,
                               ap=[[cols, 128], [256 * cols, NK2],
                                   [128 * cols, 2], [1, width or cols]])

            wq8_sb = consts.tile([128, NK2, 2, DPC], FP8)
            bq_sb = consts.tile([128, 1], F32)
            nc.sync.dma_start(out=wq8_sb[:], in_=dr_ap(wq8T, DPC))
            nc.sync.dma_start(out=bq_sb[:], in_=bq[:])
            x8_t = [None] * NKC
            x8_t[0] = state.tile([128, NK2, 2, 512], FP8, name="x8_0")
            nc.sync.dma_start(out=x8_t[0][:], in_=dr_ap(x8T, T, 0, 512))
            wk8_sb = consts.tile([128, NK2, 2, DPC], FP8)
            bk_sb = consts.tile([128, 1], F32)
            nc.sync.dma_start(out=wk8_sb[:], in_=dr_ap(wk8T, DPC))
            nc.sync.dma_start(out=bk_sb[:], in_=bk[:])
            x_t = [None] * NKC
            x8_t[1] = state.tile([128, NK2, 2, 512], FP8, name="x8_1")
            nc.sync.dma_start(out=x8_t[1][:], in_=dr_ap(x8T, T, 512, 512))
            # x tile 0 (bf16, for the V projection) in two halves
            x0a = state.tile([128, NKC // 2, 512], BF16, name="x0a")
            x0b = state.tile([128, NKC // 2, 512], BF16, name="x0b")

            def x_ap(tt, kc0, kcn):
                return bass.AP(tensor=xT.tensor,
                               offset=xT.offset + tt * 512 + kc0 * 128 * T,
                               ap=[[T, 128], [128 * T, kcn], [1, 512]])

            nc.sync.dma_start(out=x0a[:], in_=x_ap(0, 0, 4))
            wv_sb = consts.tile([128, NKC, DPC], BF16)
            bv_sb = consts.tile([128, 1], F32)
            nc.sync.dma_start(out=x0b[:], in_=x_ap(0, 4, 4))
            nc.sync.dma_start(out=wv_sb[:], in_=chunked(wvT, DPC, 0, NKC))
            nc.sync.dma_start(out=bv_sb[:], in_=bv[:])
            x_t[1] = state.tile([128, NKC, 512], BF16, name="x1")
            nc.sync.dma_start(out=x_t[1][:], in_=x_ap(1, 0, NKC))
            tri_sb = consts.tile([128, 128], BF16)
            nc.sync.dma_start(out=tri_sb[:], in_=tri[:])
            id_sb = consts.tile([128, 128], BF16)
            nc.sync.dma_start(out=id_sb[:], in_=ident[:])
            for tt in range(2, NKC):
                x8t = state.tile([128, NK2, 2, 512], FP8, name=f"x8_{tt}")
                nc.sync.dma_start(out=x8t[:], in_=dr_ap(x8T, T, tt * 512, 512))
                x8_t[tt] = x8t
                xt = state.tile([128, NKC, 512], BF16, name=f"x{tt}")
                nc.sync.dma_start(out=xt[:], in_=x_ap(tt, 0, NKC))
                x_t[tt] = xt
            wo_sb = consts.tile([128, NKC, E], BF16)
            nc.sync.dma_start(out=wo_sb[:], in_=chunked(woT, E, 0, NKC))
            selk_sb = consts.tile([16, NKC, 128], BF16)
            nc.sync.dma_start(out=selk_sb[:], in_=selk[:].rearrange(
                "k (c p) -> k c p", c=NKC))
            bo_bc = consts.tile([128, E], F32)
            nc.sync.dma_start(
                out=bo_bc[:],
                in_=bass.AP(tensor=bo.tensor, offset=bo.offset,
                            ap=[[0, 128], [1, E]]),
            )

            # ---- persistent activations -----------------------------------
            qT_sb = state.tile([128, T], BF16)   # [2-head dims, tokens]
            kT_sb = state.tile([128, T], BF16)
            vT_sb = state.tile([128, T], BF16)
            # per 128-token block: [64 v-dims, ones] per head -> the AV
            # matmul's 65-column stationary operand; the ones column makes
            # PSUM row 64 the softmax denominator.
            vN_sb = state.tile([128, T // 128, 130], BF16)
            # unnormalized ctx^T + den: rows 0-63 ctx dims, row 64 den
            ctx2_sb = state.tile([65, 2, T], BF16)

            nc.vector.memset(vN_sb[:, :, 64:65], 1.0)
            nc.vector.memset(vN_sb[:, :, 129:130], 1.0)

            # ---- stage builders -------------------------------------------
            def emit_proj(tt):
                ts = slice(tt * 512, (tt + 1) * 512)
                xa = (x0a, x0b) if tt == 0 else (x_t[tt],)
                nch = NKC // len(xa)

                ps_q = ps_p.tile([128, 512], F32, tag="p", name="ps_q")
                for kc2 in range(NK2):
                    nc.tensor.matmul(ps_q[:], wq8_sb[:, kc2, :, :],
                                     x8_t[tt][:, kc2, :, :],
                                     start=(kc2 == 0), stop=(kc2 == NK2 - 1),
                                     perf_mode=mybir.MatmulPerfMode.DoubleRow,
                                     skip_group_check=True)
                nc.vector.tensor_scalar_add(qT_sb[:, ts], ps_q[:], bq_sb[:])
                ps_k = ps_p.tile([128, 512], F32, tag="p", name="ps_k")
                for kc2 in range(NK2):
                    nc.tensor.matmul(ps_k[:], wk8_sb[:, kc2, :, :],
                                     x8_t[tt][:, kc2, :, :],
                                     start=(kc2 == 0), stop=(kc2 == NK2 - 1),
                                     perf_mode=mybir.MatmulPerfMode.DoubleRow,
                                     skip_group_check=True)
                nc.vector.tensor_scalar_add(kT_sb[:, ts], ps_k[:], bk_sb[:])
                ps_v = ps_p.tile([128, 512], F32, tag="p", name="ps_v")
                for kc in range(NKC):
                    xt = xa[kc // nch]
                    nc.tensor.matmul(ps_v[:], wv_sb[:, kc, :],
                                     xt[:, kc % nch, :],
                                     start=(kc == 0), stop=(kc == NKC - 1),
                                     skip_group_check=True)
                nc.vector.tensor_scalar_add(vT_sb[:, ts], ps_v[:], bv_sb[:])
                tp_ps = ps_p.tile([128, 4, 128], BF16, tag="p", name="tp_ps")
                for ti, tb in enumerate(range(tt * 4, tt * 4 + 4)):
                    nc.tensor.transpose(
                        tp_ps[:, ti, :], vT_sb[:, tb * 128:(tb + 1) * 128],
                        id_sb[:])
                    nc.vector.tensor_copy(vN_sb[:, tb, 0:64],
                                          tp_ps[:, ti, 0:64])
                    nc.vector.tensor_copy(vN_sb[:, tb, 65:129],
                                          tp_ps[:, ti, 64:128])

            def emit_attn(b, qt):
                t0 = b * S
                q0 = t0 + qt * 512
                nkb = 4 * qt + 4

                def emit_scores(kb):
                    c0 = max(kb - 4 * qt, 0) * 128
                    s = ps_s.tile([128, 2, 512], F32, tag="s", name="s_ps")
                    for h in range(2):
                        d0 = h * 64
                        nc.tensor.matmul(
                            s[:, h, c0:512],
                            kT_sb[d0:d0 + 64,
                                  t0 + kb * 128:t0 + (kb + 1) * 128],
                            qT_sb[d0:d0 + 64, q0 + c0:q0 + 512],
                            start=True, stop=True, skip_group_check=True)
                    return s

                s_tiles = {0: emit_scores(0)}
                cn = [ps_c.tile([128, 512], F32, tag="cn", name=f"cn{h}")
                      for h in range(2)]
                for kb in range(nkb):
                    m = kb - 4 * qt
                    c0 = max(m, 0) * 128
                    if kb + 1 < nkb:
                        s_tiles[kb + 1] = emit_scores(kb + 1)
                    s = s_tiles.pop(kb)
                    e = ep.tile([128, 2, 512], BF16, tag="e", name="e_sb")
                    # Wq and Wk are pre-scaled by 64 on the host (their
                    # 0.02-sigma values would be subnormal in fp8e4m3), so
                    # the logits carry an extra 64*64 factor undone here.
                    nc.scalar.activation(e[:, :, c0:512], s[:, :, c0:512],
                                         AFT.Exp, scale=0.125 / 4096.0)
                    if m >= 0:  # triangular block on the diagonal
                        nc.vector.tensor_mul(
                            e[:, :, c0:c0 + 128], e[:, :, c0:c0 + 128],
                            tri_sb[:].unsqueeze(1).broadcast_to((128, 2, 128)))
                    for h in range(2):
                        nc.tensor.matmul(
                            cn[h][0:65, c0:512],
                            vN_sb[:, b * SB + kb, 65 * h:65 * h + 65],
                            e[:, h, c0:512],
                            start=(kb == 0), stop=(kb == nkb - 1),
                            skip_group_check=True)

                # stage unnormalized ctx + den rows for the AllToAll
                for h in range(2):
                    nc.vector.tensor_copy(ctx2_sb[:, h, q0:q0 + 512],
                                          cn[h][0:65, :])

            def emit_half_a2a(b, hf, eng=None):
                # half-batch = query groups {2hf, 2hf+1}; peer j gets this
                # core's heads' ctx for j's 128 half-batch tokens.
                # eng picks the queue that hosts the (blocking) collective
                # trigger: GpSimd normally; the LAST collective goes on
                # Scalar (idle once exp ends) so its trigger doesn't queue
                # behind the previous collective's completion wait.
                eng = eng or nc.gpsimd
                base = b * S + hf * (S // 2)
                ctxd = dram.tile([N_CORES, CR, PH], BF16, tag="ctxd",
                                 name="ctxd", bufs=4)
                for h in range(2):
                    nc.sync.dma_start(
                        out=bass.AP(tensor=ctxd.tensor,
                                    offset=ctxd[0].offset + h * 65 * PH,
                                    ap=[[PH, 65], [CR * PH, N_CORES],
                                        [1, PH]]),
                        in_=ctx2_sb[:, h, base:base + S // 2].rearrange(
                            "p (j t) -> p j t", j=N_CORES))
                recv = dram.tile([N_CORES, CR, PH], BF16, tag="recv",
                                 name="recv", bufs=4)
                bass.BassGpSimd.collective_compute(
                    eng,
                    "AllToAll",
                    mybir.AluOpType.bypass,
                    replica_groups=[list(range(N_CORES))],
                    ins=[ctxd.opt()],
                    outs=[recv.opt()],
                )
                return recv

            def emit_half_recv(b, hf, recv, eng=None):
                # gather + normalize the received ctx for one half-batch.
                # eng picks the DMA issue queue: Sync mid-kernel, Scalar for
                # the tail chains (ACT is idle there, Sync is not).
                eng = eng or nc.sync
                cg_sb = op.tile([128, NKC, PH], BF16, tag="cg_sb", name="cg_sb",
                                bufs=2)
                den16 = op.tile([16, PH], BF16, tag="den16", name="den16",
                                bufs=2)
                r0 = recv[0]
                # den16 gather issues FIRST so the reciprocal (the head of
                # the broadcast chain) starts while cg still transfers
                eng.dma_start(
                    out=den16[:],
                    in_=bass.AP(tensor=r0.tensor,
                                offset=r0.offset + 64 * PH,
                                ap=[[CR * PH, N_CORES], [65 * PH, 2],
                                    [1, PH]]))
                for h in range(2):
                    eng.dma_start(
                        out=cg_sb[h * 64:(h + 1) * 64, :, :],
                        in_=bass.AP(tensor=r0.tensor,
                                    offset=r0.offset + h * 65 * PH,
                                    ap=[[PH, 64], [CR * PH, N_CORES],
                                        [1, PH]]))
                r16 = op.tile([16, PH], BF16, tag="r16", name="r16", bufs=2)
                with nc.allow_low_precision(
                        reason="bf16 softmax scale, |rel err| ~4e-3 ok"):
                    nc.vector.reciprocal(r16[:], den16[:])
                # broadcast r16 to the cg layout with 8 K=16 matmuls:
                # rmap_ps[p, kc, t] = r16[2*kc + p//64, t]; also nudges the
                # PE awake right before the Wo block that follows.
                rmap_ps = ps_s.tile([128, 2, 512], F32, tag="s",
                                    name="rmap_ps")
                for kc in range(NKC):
                    nc.tensor.matmul(
                        rmap_ps[:, kc >> 2,
                                (kc & 3) * 128:((kc & 3) + 1) * 128],
                        selk_sb[:, kc, :], r16[:],
                        start=True, stop=True, skip_group_check=True)
                nc.vector.tensor_mul(
                    cg_sb[:].rearrange("p a b -> p (a b)"),
                    cg_sb[:].rearrange("p a b -> p (a b)"),
                    rmap_ps[:].rearrange("p a b -> p (a b)"))
                return b, hf, cg_sb

            def emit_half_wo(b, hf, cg_sb):
                o_sb = op.tile([PH, E], F32, tag="o_sb", name="o_sb", bufs=3)
                for et in range(2):
                    ps = ps_s.tile([128, 2, 512], F32, tag="s", name="c_ps")
                    for kc in range(NKC):
                        nc.tensor.matmul(
                            ps[0:PH, 0, :],
                            cg_sb[:, kc, :],
                            wo_sb[:, kc, et * 512:(et + 1) * 512],
                            start=(kc == 0), stop=(kc == NKC - 1),
                            skip_group_check=True)
                    nc.vector.tensor_add(
                        o_sb[:, et * 512:(et + 1) * 512], ps[0:PH, 0, :],
                        bo_bc[0:PH, et * 512:(et + 1) * 512])
                    r0w = (b * 2 + hf) * PH
                    nc.sync.dma_start(
                        out=out[r0w:r0w + PH, et * 512:(et + 1) * 512],
                        in_=o_sb[:, et * 512:(et + 1) * 512])

            # ---- schedule -------------------------------------------------
            # qt order {2,3,0,1} per batch: half 1 (28 key-blocks of work)
            # finishes first and its a2a fires ~70 us before the end; half 0
            # ({q0,q1}, only 12 key-blocks) finishes last so the FINAL
            # collective fires ~15 us of attention + ~15 us of held-back Wo
            # work before the PE drains.  4 collectives, spaced >=15 us
            # (back-to-back a2as degrade ~3x on this part) and the first
            # completes no earlier than ~100 us (first-call barrier), so
            # every collective-dependent Wo block is placed with >=20 us of
            # slack after its gating collective's expected completion.
            emit_proj(0)
            emit_proj(1)
            emit_proj(2)
            # -------- batch 0
            emit_attn(0, 2)
            emit_proj(3)
            emit_attn(0, 3)
            rB = emit_half_a2a(0, 1)          # cc1, trigger ~55us
            emit_attn(0, 0)
            emit_proj(4)
            emit_attn(0, 1)
            rA = emit_half_a2a(0, 0)          # cc2, trigger ~75us
            emit_proj(5)
            emit_proj(6)
            # -------- batch 1
            emit_attn(1, 2)
            emit_proj(7)
            emit_attn(1, 3)
            rD = emit_half_a2a(1, 1)          # cc3, trigger ~125us
            # recv chains emit AFTER the a2a staging so their cc-gated
            # gather DMAs can never delay staging on the in-order Sync queue
            args01 = emit_half_recv(0, 1, rB)  # cc1 done ~105us
            emit_half_wo(*args01)             # PE ~130us
            emit_attn(1, 0)
            emit_attn(1, 1)
            # (a Scalar-queue collective trigger fails at NEFF load time -
            # all collectives stay on GpSimd; tail recv gathers on Sync in
            # cc-completion order)
            rC = emit_half_a2a(1, 0)          # cc4 (tail), trigger ~150us
            # tail: independent Wo blocks keep the PE busy through the
            # final collective + its recv chain
            args00 = emit_half_recv(0, 0, rA)  # cc2 done ~110us
            emit_half_wo(*args00)
            args11 = emit_half_recv(1, 1, rD)  # cc3 done ~160us
            emit_half_wo(*args11)
            args10 = emit_half_recv(1, 0, rC)
            emit_half_wo(*args10)

    nc.compile()
    return nc


_NC = None


def _get_program():
    global _NC
    if _NC is None:
        _NC = build_program()
    return _NC


def _bf(a):
    return np.ascontiguousarray(a).astype(ml_dtypes.bfloat16)


def _f8(a):
    return np.ascontiguousarray(a).astype(ml_dtypes.float8_e4m3)


def kernel(x, Wq, bq, Wk, bk, Wv, bv, Wo, bo, _trace=False, _trace_kwargs=None):
    x = np.asarray(x, np.float32)
    Wq, Wk, Wv, Wo = (np.asarray(w, np.float32) for w in (Wq, Wk, Wv, Wo))
    bq, bk, bv, bo = (np.asarray(v, np.float32) for v in (bq, bk, bv, bo))

    xf = x.reshape(T, E).T
    xT = _bf(xf)
    x8T = _f8(xf)
    i = np.arange(128)
    tri = _bf((i[:, None] <= i[None, :]).astype(np.float32))
    ident = _bf(np.eye(128, dtype=np.float32))
    k16 = np.arange(16)[:, None]
    kcp = (2 * (np.arange(NKC * 128) // 128) + (np.arange(NKC * 128) % 128) // 64)
    selk = _bf((k16 == kcp[None, :]).astype(np.float32))

    in_maps = []
    for c in range(N_CORES):
        sl = slice(c * DPC, (c + 1) * DPC)
        in_maps.append({
            "xT": xT,
            "x8T": x8T,
            "wq8T": _f8(64.0 * Wq[sl, :].T),
            "wk8T": _f8(64.0 * Wk[sl, :].T),
            "wvT": _bf(Wv[sl, :].T),
            "woT": _bf(Wo.T),
            "bq": 64.0 * bq[sl].reshape(DPC, 1),
            "bk": 64.0 * bk[sl].reshape(DPC, 1),
            "bv": bv[sl].reshape(DPC, 1).copy(),
            "bo": bo,
            "tri": tri,
            "ident": ident,
            "selk": selk,
        })

    nc = _get_program()
    res = run_bass_kernel_spmd(nc, in_maps, list(range(N_CORES)),
                               trace=_trace, **(_trace_kwargs or {}))
    # out[c] rows are [batch, half, 128]: row (b, hf, r) holds global
    # token b*2048 + hf*1024 + c*128 + r.
    stacked = np.stack([res.results[i]["out"].reshape(B, 2, PH, E)
                        for i in range(N_CORES)], axis=2)
    full = stacked.reshape(T, E)
    if _trace:
        return full.reshape(B, S, E), res
    return full.reshape(B, S, E)


# revision 45
# speedup vs baseline: 1.0619x; 1.0619x over previous
"""Multi-head attention (B=2, S=2048, H=16, D=64) on 8 Trainium2 NeuronCores.

Head-parallel tensor parallelism: core c owns heads {2c, 2c+1} (a 128-dim
slice of the model dim): column-parallel QKV projections and local causal
attention for its 2 heads, then an AllToAll of bf16 context vectors (one
512-token query group at a time) and a full-width Wo projection for this
core's own disjoint 64-token output slices.

Schedule (v6), shaped by trace measurements:

* q/k projections run in fp8e4m3 with perf_mode=DoubleRow (256-wide
  contraction chunks, ~1.5x PE throughput); Wq/Wk are pre-scaled by 64 on
  the host (their 0.02-sigma entries would land in e4m3's subnormal range)
  and the 64*64 logit factor is undone inside the exp's free scale.  The
  V projection stays bf16: early-token v errors are unprotected by
  softmax averaging and would breach the error budget.
* Startup: wq8/bq and the fp8 x tile 0 are the first DMAs issued;
  everything else loads behind them in deadline order (tri before the
  first diagonal block, wo/bo last).
* Query groups run in order {2, 3, 0, 1} per batch so the LAST collective
  (half 0, only 12 key-blocks of attention) fires well before the PE
  drains.  4 collectives spaced >=15 us: per-qt granularity (8
  collectives) was measured to DEGRADE - back-to-back AllToAlls on this
  part grow from ~6 us to ~22 us each - and the first collective
  completes no earlier than ~100 us (first-call barrier), so every
  collective-dependent Wo block is placed with >=20 us of slack and the
  last Wo blocks are held back as PE filler for the final collective.
* PSUM: proj pool 2 banks (q/k/v/transpose rotate through [128,512] slots),
  scores 2x2 banks, ctx accumulators 2 banks = exactly 8.  The projection
  for tile t+1 and the Wo matmuls are emitted after the attention section
  that hides them; the Tile scheduler slots them into the PE stalls where
  attention waits on the ACT exp stream (exp on ACT is ~1.15 us per
  128-key block vs ~0.65 us of PE work, so without filler the PE idles
  ~40% during attention and the HAM clock gate re-throttles).
* Softmax normalization happens on the receiving core (the a2a payload is
  65 rows per head: 64 unnormalized ctx dims + the denominator row from a
  trailing ones-column in the AV stationary); 16 denominator rows stack on
  the partition axis at the receiver: a 16-lane DVE reciprocal, then 8
  K=16 one-hot selector matmuls broadcast the reciprocals into PSUM in cg
  layout (replacing a DRAM-bounced stride-0 DMA round trip, and nudging
  the PE awake right before the Wo block), then one fused DVE multiply
  normalizes the gathered ctx.  Tail recv chains issue their gather DMAs
  from the Scalar queue (idle in the tail; Sync is not).
* Attention-times-V keeps V plus a trailing ones column as the 65-column
  stationary operand and streams the exp tile; scores use tile_position
  row pairs so the two heads' score matmuls run concurrently; exp is one
  ACT instruction per key block covering both heads; the diagonal tri-mask
  is one DVE multiply per block via a stride-0 broadcast AP over heads.
* A tiny warm-up AllToAll is issued during the load phase so the first real
  collective doesn't pay the ~23 us first-call setup on the critical path.
"""

import sys

sys.path.insert(0, "/opt/trn_rl_repo")

import ml_dtypes
import numpy as np

import concourse.bass as bass
import concourse.tile as tile
from concourse import bacc, mybir
from concourse.bass_utils import run_bass_kernel_spmd

N_CORES = 8
B, S, H, D = 2, 2048, 16, 64
E = H * D            # 1024
T = B * S            # 4096 tokens
DPC = 128            # dims (2 heads) per core
NKC = E // 128       # 8 contraction chunks for the projections
SB = S // 128        # 16 key blocks per batch
PHQ = 512 // N_CORES  # 64 tokens per core per query group
PH = 2 * PHQ         # 128 tokens per core per half-batch
CR = 130             # a2a chunk rows: 2 x (64 ctx dims + den)

F32 = mybir.dt.float32
BF16 = mybir.dt.bfloat16
FP8 = mybir.dt.float8e4
AFT = mybir.ActivationFunctionType
NK2 = 4              # 256-wide contraction chunks for DoubleRow q/k

QT_ORDER = (0, 1, 3, 2)  # hf0 = {0,1} finishes early; q2 (12 blocks) last


def build_program():
    nc = bacc.Bacc("TRN2", target_bir_lowering=False, debug=False,
                   num_devices=N_CORES)

    xT = nc.dram_tensor("xT", [E, T], BF16, kind="ExternalInput").ap()
    # fp8 copies of x and Wq/Wk for DoubleRow q/k projections (v stays
    # bf16: early-token v errors are unprotected by softmax averaging)
    x8T = nc.dram_tensor("x8T", [E, T], FP8, kind="ExternalInput").ap()
    wq8T = nc.dram_tensor("wq8T", [E, DPC], FP8, kind="ExternalInput").ap()
    wk8T = nc.dram_tensor("wk8T", [E, DPC], FP8, kind="ExternalInput").ap()
    wvT = nc.dram_tensor("wvT", [E, DPC], BF16, kind="ExternalInput").ap()
    woT = nc.dram_tensor("woT", [E, E], BF16, kind="ExternalInput").ap()
    bq = nc.dram_tensor("bq", [DPC, 1], F32, kind="ExternalInput").ap()
    bk = nc.dram_tensor("bk", [DPC, 1], F32, kind="ExternalInput").ap()
    bv = nc.dram_tensor("bv", [DPC, 1], F32, kind="ExternalInput").ap()
    bo = nc.dram_tensor("bo", [E], F32, kind="ExternalInput").ap()
    # single 128x128 lower-triangular (k_local <= q_local) mask
    tri = nc.dram_tensor("tri", [128, 128], BF16, kind="ExternalInput").ap()
    ident = nc.dram_tensor("ident", [128, 128], BF16, kind="ExternalInput").ap()
    # selk[k, kc*128+p] = 1 iff k == 2*kc + p//64: one-hot selectors that
    # broadcast the 16 per-(peer,head) reciprocal rows to the cg layout
    # via 8 tiny K=16 matmuls (replaces a DRAM-bounced stride-0 DMA).
    selk = nc.dram_tensor("selk", [16, NKC * 128], BF16,
                          kind="ExternalInput").ap()
    out = nc.dram_tensor("out", [T // N_CORES, E], F32, kind="ExternalOutput").ap()

    with tile.TileContext(nc) as tc:
        with (
            tc.tile_pool(name="consts", bufs=1) as consts,
            tc.tile_pool(name="state", bufs=1) as state,
            tc.tile_pool(name="ep", bufs=6) as ep,
            tc.tile_pool(name="op", bufs=4) as op,
            tc.tile_pool(name="ps_p", bufs=2, space="PSUM") as ps_p,
            tc.tile_pool(name="ps_s", bufs=2, space="PSUM") as ps_s,
            tc.tile_pool(name="ps_c", bufs=2, space="PSUM") as ps_c,
            tc.tile_pool(name="dram", bufs=1, space="DRAM") as dram,
        ):
            # ---- warm-up collective: absorbs the first-AllToAll setup cost
            # while the DMA engines are still loading x ----------------------
            wu_s = consts.tile([128, 16], BF16)
            nc.vector.memset(wu_s[:], 0.0)
            wu_in = dram.tile([N_CORES, 16, 16], BF16, tag="wu_in", name="wu_in")
            wu_out = dram.tile([N_CORES, 16, 16], BF16, tag="wu_out",
                               name="wu_out")
            nc.sync.dma_start(out=wu_in[:], in_=wu_s[:])
            nc.gpsimd.collective_compute(
                "AllToAll",
                mybir.AluOpType.bypass,
                replica_groups=[list(range(N_CORES))],
                ins=[wu_in.opt()],
                outs=[wu_out.opt()],
            )

            def chunked(dram_ap, cols, kc0, kcn):
                # DRAM [E, cols] viewed as [p, kc, cols]: row kc*128+p
                return bass.AP(tensor=dram_ap.tensor,
                               offset=dram_ap.offset + kc0 * 128 * cols,
                               ap=[[cols, 128], [128 * cols, kcn], [1, cols]])

            # ---- loads in deadline order ---------------------------------
            # fp8 weights/x in DoubleRow layout: contraction index
            # k = kc2*256 + i*128 + p -> [p, kc2, i, cols]
            def dr_ap(dram_ap, cols, toff=0, width=None):
                return bass.AP(tensor=dram_ap.tensor,
                               offset=dram_ap.offset + toff,
                               ap=[[cols, 128], [256 * cols, NK2],
                                   [128 * cols, 2], [1, width or cols]])

            wq8_sb = consts.tile([128, NK2, 2, DPC], FP8)
            bq_sb = consts.tile([128, 1], F32)
            nc.sync.dma_start(out=wq8_sb[:], in_=dr_ap(wq8T, DPC))
            nc.sync.dma_start(out=bq_sb[:], in_=bq[:])
            x8_t = [None] * NKC
            x8_t[0] = state.tile([128, NK2, 2, 512], FP8, name="x8_0")
            nc.sync.dma_start(out=x8_t[0][:], in_=dr_ap(x8T, T, 0, 512))
            wk8_sb = consts.tile([128, NK2, 2, DPC], FP8)
            bk_sb = consts.tile([128, 1], F32)
            nc.sync.dma_start(out=wk8_sb[:], in_=dr_ap(wk8T, DPC))
            nc.sync.dma_start(out=bk_sb[:], in_=bk[:])
            x_t = [None] * NKC
            x8_t[1] = state.tile([128, NK2, 2, 512], FP8, name="x8_1")
            nc.sync.dma_start(out=x8_t[1][:], in_=dr_ap(x8T, T, 512, 512))
            # x tile 0 (bf16, for the V projection) in two halves
            x0a = state.tile([128, NKC // 2, 512], BF16, name="x0a")
            x0b = state.tile([128, NKC // 2, 512], BF16, name="x0b")

            def x_ap(tt, kc0, kcn):
                return bass.AP(tensor=xT.tensor,
                               offset=xT.offset + tt * 512 + kc0 * 128 * T,
                               ap=[[T, 128], [128 * T, kcn], [1, 512]])

            nc.sync.dma_start(out=x0a[:], in_=x_ap(0, 0, 4))
            wv_sb = consts.tile([128, NKC, DPC], BF16)
            bv_sb = consts.tile([128, 1], F32)
            nc.sync.dma_start(out=x0b[:], in_=x_ap(0, 4, 4))
            nc.sync.dma_start(out=wv_sb[:], in_=chunked(wvT, DPC, 0, NKC))
            x8_t[2] = state.tile([128, NK2, 2, 512], FP8, name="x8_2")
            nc.sync.dma_start(out=x8_t[2][:], in_=dr_ap(x8T, T, 1024, 512))
            nc.sync.dma_start(out=bv_sb[:], in_=bv[:])
            x_t[1] = state.tile([128, NKC, 512], BF16, name="x1")
            nc.sync.dma_start(out=x_t[1][:], in_=x_ap(1, 0, NKC))
            tri_sb = consts.tile([128, 128], BF16)
            nc.sync.dma_start(out=tri_sb[:], in_=tri[:])
            id_sb = consts.tile([128, 128], BF16)
            nc.sync.dma_start(out=id_sb[:], in_=ident[:])
            x_t[2] = state.tile([128, NKC, 512], BF16, name="x2")
            nc.sync.dma_start(out=x_t[2][:], in_=x_ap(2, 0, NKC))
            for tt in range(3, NKC):
                x8t = state.tile([128, NK2, 2, 512], FP8, name=f"x8_{tt}")
                nc.sync.dma_start(out=x8t[:], in_=dr_ap(x8T, T, tt * 512, 512))
                x8_t[tt] = x8t
                xt = state.tile([128, NKC, 512], BF16, name=f"x{tt}")
                nc.sync.dma_start(out=xt[:], in_=x_ap(tt, 0, NKC))
                x_t[tt] = xt
            wo_sb = consts.tile([128, NKC, E], BF16)
            nc.sync.dma_start(out=wo_sb[:], in_=chunked(woT, E, 0, NKC))
            selk_sb = consts.tile([16, NKC, 128], BF16)
            nc.sync.dma_start(out=selk_sb[:], in_=selk[:].rearrange(
                "k (c p) -> k c p", c=NKC))
            bo_bc = consts.tile([128, E], F32)
            nc.sync.dma_start(
                out=bo_bc[:],
                in_=bass.AP(tensor=bo.tensor, offset=bo.offset,
                            ap=[[0, 128], [1, E]]),
            )

            # ---- persistent activations -----------------------------------
            qT_sb = state.tile([128, T], BF16)   # [2-head dims, tokens]
            kT_sb = state.tile([128, T], BF16)
            vT_sb = state.tile([128, T], BF16)
            # per 128-token block: [64 v-dims, ones] per head -> the AV
            # matmul's 65-column stationary operand; the ones column makes
            # PSUM row 64 the softmax denominator.
            vN_sb = state.tile([128, T // 128, 130], BF16)
            # unnormalized ctx^T + den: rows 0-63 ctx dims, row 64 den
            ctx2_sb = state.tile([65, 2, T], BF16)

            nc.vector.memset(vN_sb[:, :, 64:65], 1.0)
            nc.vector.memset(vN_sb[:, :, 129:130], 1.0)

            # ---- stage builders -------------------------------------------
            def emit_proj_qk(tt):
                # fp8 q/k projections only - gate the score stream on just
                # the (small) fp8 x tiles so attention starts early
                ts = slice(tt * 512, (tt + 1) * 512)
                ps_q = ps_p.tile([128, 512], F32, tag="p", name="ps_q")
                for kc2 in range(NK2):
                    nc.tensor.matmul(ps_q[:], wq8_sb[:, kc2, :, :],
                                     x8_t[tt][:, kc2, :, :],
                                     start=(kc2 == 0), stop=(kc2 == NK2 - 1),
                                     perf_mode=mybir.MatmulPerfMode.DoubleRow,
                                     skip_group_check=True)
                nc.vector.tensor_scalar_add(qT_sb[:, ts], ps_q[:], bq_sb[:])
                ps_k = ps_p.tile([128, 512], F32, tag="p", name="ps_k")
                for kc2 in range(NK2):
                    nc.tensor.matmul(ps_k[:], wk8_sb[:, kc2, :, :],
                                     x8_t[tt][:, kc2, :, :],
                                     start=(kc2 == 0), stop=(kc2 == NK2 - 1),
                                     perf_mode=mybir.MatmulPerfMode.DoubleRow,
                                     skip_group_check=True)
                nc.vector.tensor_scalar_add(kT_sb[:, ts], ps_k[:], bk_sb[:])

            def emit_proj_v(tt):
                # bf16 V projection + transposes, deferred to each tile's
                # first AV deadline (paced by the bf16 x tile loads)
                ts = slice(tt * 512, (tt + 1) * 512)
                xa = (x0a, x0b) if tt == 0 else (x_t[tt],)
                nch = NKC // len(xa)
                ps_v = ps_p.tile([128, 512], F32, tag="p", name="ps_v")
                for kc in range(NKC):
                    xt = xa[kc // nch]
                    nc.tensor.matmul(ps_v[:], wv_sb[:, kc, :],
                                     xt[:, kc % nch, :],
                                     start=(kc == 0), stop=(kc == NKC - 1),
                                     skip_group_check=True)
                nc.vector.tensor_scalar_add(vT_sb[:, ts], ps_v[:], bv_sb[:])
                tp_ps = ps_p.tile([128, 4, 128], BF16, tag="p", name="tp_ps")
                for ti, tb in enumerate(range(tt * 4, tt * 4 + 4)):
                    nc.tensor.transpose(
                        tp_ps[:, ti, :], vT_sb[:, tb * 128:(tb + 1) * 128],
                        id_sb[:])
                    nc.vector.tensor_copy(vN_sb[:, tb, 0:64],
                                          tp_ps[:, ti, 0:64])
                    nc.vector.tensor_copy(vN_sb[:, tb, 65:129],
                                          tp_ps[:, ti, 64:128])

            def emit_proj(tt):
                emit_proj_qk(tt)
                emit_proj_v(tt)

            def emit_attn(b, qt):
                t0 = b * S
                q0 = t0 + qt * 512
                nkb = 4 * qt + 4

                def emit_scores(kb):
                    c0 = max(kb - 4 * qt, 0) * 128
                    s = ps_s.tile([128, 2, 512], F32, tag="s", name="s_ps")
                    for h in range(2):
                        d0 = h * 64
                        nc.tensor.matmul(
                            s[:, h, c0:512],
                            kT_sb[d0:d0 + 64,
                                  t0 + kb * 128:t0 + (kb + 1) * 128],
                            qT_sb[d0:d0 + 64, q0 + c0:q0 + 512],
                            start=True, stop=True, skip_group_check=True)
                    return s

                s_tiles = {0: emit_scores(0)}
                cn = [ps_c.tile([128, 512], F32, tag="cn", name=f"cn{h}")
                      for h in range(2)]
                for kb in range(nkb):
                    m = kb - 4 * qt
                    c0 = max(m, 0) * 128
                    if kb + 1 < nkb:
                        s_tiles[kb + 1] = emit_scores(kb + 1)
                    s = s_tiles.pop(kb)
                    e = ep.tile([128, 2, 512], BF16, tag="e", name="e_sb")
                    # Wq and Wk are pre-scaled by 64 on the host (their
                    # 0.02-sigma values would be subnormal in fp8e4m3), so
                    # the logits carry an extra 64*64 factor undone here.
                    nc.scalar.activation(e[:, :, c0:512], s[:, :, c0:512],
                                         AFT.Exp, scale=0.125 / 4096.0)
                    if m >= 0:  # triangular block on the diagonal
                        nc.vector.tensor_mul(
                            e[:, :, c0:c0 + 128], e[:, :, c0:c0 + 128],
                            tri_sb[:].unsqueeze(1).broadcast_to((128, 2, 128)))
                    for h in range(2):
                        nc.tensor.matmul(
                            cn[h][0:65, c0:512],
                            vN_sb[:, b * SB + kb, 65 * h:65 * h + 65],
                            e[:, h, c0:512],
                            start=(kb == 0), stop=(kb == nkb - 1),
                            skip_group_check=True)

                # stage unnormalized ctx + den rows for the AllToAll
                for h in range(2):
                    nc.vector.tensor_copy(ctx2_sb[:, h, q0:q0 + 512],
                                          cn[h][0:65, :])

            def emit_half_a2a(b, hf):
                # half-batch = query groups {2hf, 2hf+1}; peer j gets this
                # core's heads' ctx for j's 128 half-batch tokens.
                base = b * S + hf * (S // 2)
                ctxd = dram.tile([N_CORES, CR, PH], BF16, tag="ctxd",
                                 name="ctxd", bufs=4)
                for h in range(2):
                    nc.sync.dma_start(
                        out=bass.AP(tensor=ctxd.tensor,
                                    offset=ctxd[0].offset + h * 65 * PH,
                                    ap=[[PH, 65], [CR * PH, N_CORES],
                                        [1, PH]]),
                        in_=ctx2_sb[:, h, base:base + S // 2].rearrange(
                            "p (j t) -> p j t", j=N_CORES))
                recv = dram.tile([N_CORES, CR, PH], BF16, tag="recv",
                                 name="recv", bufs=4)
                nc.gpsimd.collective_compute(
                    "AllToAll",
                    mybir.AluOpType.bypass,
                    replica_groups=[list(range(N_CORES))],
                    ins=[ctxd.opt()],
                    outs=[recv.opt()],
                )
                return recv

            def emit_half_recv(b, hf, recv, eng=None):
                # gather + normalize the received ctx for one half-batch.
                # eng picks the DMA issue queue: Sync mid-kernel, Scalar for
                # the tail chains (ACT is idle there, Sync is not).
                eng = eng or nc.sync
                cg_sb = op.tile([128, NKC, PH], BF16, tag="cg_sb", name="cg_sb",
                                bufs=2)
                den16 = op.tile([16, PH], BF16, tag="den16", name="den16",
                                bufs=2)
                r0 = recv[0]
                # den16 gathers first: the reciprocal heads the broadcast
                # chain, so it starts while cg still transfers
                eng.dma_start(
                    out=den16[:],
                    in_=bass.AP(tensor=r0.tensor,
                                offset=r0.offset + 64 * PH,
                                ap=[[CR * PH, N_CORES], [65 * PH, 2],
                                    [1, PH]]))
                for h in range(2):
                    eng.dma_start(
                        out=cg_sb[h * 64:(h + 1) * 64, :, :],
                        in_=bass.AP(tensor=r0.tensor,
                                    offset=r0.offset + h * 65 * PH,
                                    ap=[[PH, 64], [CR * PH, N_CORES],
                                        [1, PH]]))
                r16 = op.tile([16, PH], BF16, tag="r16", name="r16", bufs=2)
                with nc.allow_low_precision(
                        reason="bf16 softmax scale, |rel err| ~4e-3 ok"):
                    nc.vector.reciprocal(r16[:], den16[:])
                # broadcast r16 to the cg layout with 8 K=16 matmuls:
                # rmap_ps[p, kc, t] = r16[2*kc + p//64, t]; also nudges the
                # PE awake right before the Wo block that follows.
                rmap_ps = ps_s.tile([128, 2, 512], F32, tag="s",
                                    name="rmap_ps")
                for kc in range(NKC):
                    nc.tensor.matmul(
                        rmap_ps[:, kc >> 2,
                                (kc & 3) * 128:((kc & 3) + 1) * 128],
                        selk_sb[:, kc, :], r16[:],
                        start=True, stop=True, skip_group_check=True)
                nc.vector.tensor_mul(
                    cg_sb[:].rearrange("p a b -> p (a b)"),
                    cg_sb[:].rearrange("p a b -> p (a b)"),
                    rmap_ps[:].rearrange("p a b -> p (a b)"))
                return b, hf, cg_sb

            def emit_half_wo(b, hf, cg_sb):
                o_sb = op.tile([PH, E], F32, tag="o_sb", name="o_sb", bufs=3)
                for et in range(2):
                    ps = ps_s.tile([128, 2, 512], F32, tag="s", name="c_ps")
                    for kc in range(NKC):
                        nc.tensor.matmul(
                            ps[0:PH, 0, :],
                            cg_sb[:, kc, :],
                            wo_sb[:, kc, et * 512:(et + 1) * 512],
                            start=(kc == 0), stop=(kc == NKC - 1),
                            skip_group_check=True)
                    nc.vector.tensor_add(
                        o_sb[:, et * 512:(et + 1) * 512], ps[0:PH, 0, :],
                        bo_bc[0:PH, et * 512:(et + 1) * 512])
                    r0w = (b * 2 + hf) * PH
                    nc.sync.dma_start(
                        out=out[r0w:r0w + PH, et * 512:(et + 1) * 512],
                        in_=o_sb[:, et * 512:(et + 1) * 512])

            # ---- schedule -------------------------------------------------
            # qt order {2,3,0,1} per batch: half 1 (28 key-blocks of work)
            # finishes first and its a2a fires ~70 us before the end; half 0
            # ({q0,q1}, only 12 key-blocks) finishes last so the FINAL
            # collective fires ~15 us of attention + ~15 us of held-back Wo
            # work before the PE drains.  4 collectives, spaced >=15 us
            # (back-to-back a2as degrade ~3x on this part) and the first
            # completes no earlier than ~100 us (first-call barrier), so
            # every collective-dependent Wo block is placed with >=20 us of
            # slack after its gating collective's expected completion.
            emit_proj_qk(0)
            emit_proj_qk(1)
            emit_proj_qk(2)
            emit_proj_v(0)
            # -------- batch 0
            emit_attn(0, 2)
            emit_proj_v(1)
            emit_proj_v(2)
            emit_proj(3)
            emit_attn(0, 3)
            rB = emit_half_a2a(0, 1)          # cc1, trigger ~55us
            emit_attn(0, 0)
            emit_proj(4)
            emit_attn(0, 1)
            rA = emit_half_a2a(0, 0)          # cc2, trigger ~75us
            emit_proj(5)
            emit_proj(6)
            # -------- batch 1
            emit_attn(1, 2)
            emit_proj(7)
            emit_attn(1, 3)
            rD = emit_half_a2a(1, 1)          # cc3, trigger ~125us
            # recv chains emit AFTER the a2a staging so their cc-gated
            # gather DMAs can never delay staging on the in-order Sync queue
            args01 = emit_half_recv(0, 1, rB)  # cc1 done ~105us
            emit_half_wo(*args01)             # PE ~130us
            emit_attn(1, 0)
            emit_attn(1, 1)
            rC = emit_half_a2a(1, 0)          # cc4 (tail), trigger ~142us
            # tail: independent Wo blocks keep the PE busy through the
            # final collective + its recv chain
            args00 = emit_half_recv(0, 0, rA)  # cc2 done ~110us
            emit_half_wo(*args00)
            args11 = emit_half_recv(1, 1, rD, eng=nc.scalar)  # cc3 ~137us
            emit_half_wo(*args11)
            args10 = emit_half_recv(1, 0, rC, eng=nc.scalar)
            emit_half_wo(*args10)

    nc.compile()
    return nc


_NC = None


def _get_program():
    global _NC
    if _NC is None:
        _NC = build_program()
    return _NC


def _bf(a):
    return np.ascontiguousarray(a).astype(ml_dtypes.bfloat16)


def _f8(a):
    return np.ascontiguousarray(a).astype(ml_dtypes.float8_e4m3)


def kernel(x, Wq, bq, Wk, bk, Wv, bv, Wo, bo, _trace=False, _trace_kwargs=None):
    x = np.asarray(x, np.float32)
    Wq, Wk, Wv, Wo = (np.asarray(w, np.float32) for w in (Wq, Wk, Wv, Wo))
    bq, bk, bv, bo = (np.asarray(v, np.float32) for v in (bq, bk, bv, bo))

    xf = x.reshape(T, E).T
    xT = _bf(xf)
    x8T = _f8(xf)
    i = np.arange(128)
    tri = _bf((i[:, None] <= i[None, :]).astype(np.float32))
    ident = _bf(np.eye(128, dtype=np.float32))
    k16 = np.arange(16)[:, None]
    kcp = (2 * (np.arange(NKC * 128) // 128) + (np.arange(NKC * 128) % 128) // 64)
    selk = _bf((k16 == kcp[None, :]).astype(np.float32))

    in_maps = []
    for c in range(N_CORES):
        sl = slice(c * DPC, (c + 1) * DPC)
        in_maps.append({
            "xT": xT,
            "x8T": x8T,
            "wq8T": _f8(64.0 * Wq[sl, :].T),
            "wk8T": _f8(64.0 * Wk[sl, :].T),
            "wvT": _bf(Wv[sl, :].T),
            "woT": _bf(Wo.T),
            "bq": 64.0 * bq[sl].reshape(DPC, 1),
            "bk": 64.0 * bk[sl].reshape(DPC, 1),
            "bv": bv[sl].reshape(DPC, 1).copy(),
            "bo": bo,
            "tri": tri,
            "ident": ident,
            "selk": selk,
        })

    nc = _get_program()
    res = run_bass_kernel_spmd(nc, in_maps, list(range(N_CORES)),
                               trace=_trace, **(_trace_kwargs or {}))
    # out[c] rows are [batch, half, 128]: row (b, hf, r) holds global
    # token b*2048 + hf*1024 + c*128 + r.
    stacked = np.stack([res.results[i]["out"].reshape(B, 2, PH, E)
                        for i in range(N_CORES)], axis=2)
    full = stacked.reshape(T, E)
    if _trace:
        return full.reshape(B, S, E), res
    return full.reshape(B, S, E)


# revision 47
# speedup vs baseline: 1.0986x; 1.0346x over previous
"""Multi-head attention (B=2, S=2048, H=16, D=64) on 8 Trainium2 NeuronCores.

Head-parallel tensor parallelism: core c owns heads {2c, 2c+1} (a 128-dim
slice of the model dim): column-parallel QKV projections and local causal
attention for its 2 heads, then an AllToAll of bf16 context vectors (one
512-token query group at a time) and a full-width Wo projection for this
core's own disjoint 64-token output slices.

Schedule (v6), shaped by trace measurements:

* q/k projections run in fp8e4m3 with perf_mode=DoubleRow (256-wide
  contraction chunks, ~1.5x PE throughput); Wq/Wk are pre-scaled by 64 on
  the host (their 0.02-sigma entries would land in e4m3's subnormal range)
  and the 64*64 logit factor is undone inside the exp's free scale.  The
  V projection stays bf16: early-token v errors are unprotected by
  softmax averaging and would breach the error budget.
* Startup: wq8/bq and the fp8 x tile 0 are the first DMAs issued;
  everything else loads behind them in deadline order (tri before the
  first diagonal block, wo/bo last).
* Query groups run in order {2, 3, 0, 1} per batch so the LAST collective
  (half 0, only 12 key-blocks of attention) fires well before the PE
  drains.  4 collectives spaced >=15 us: per-qt granularity (8
  collectives) was measured to DEGRADE - back-to-back AllToAlls on this
  part grow from ~6 us to ~22 us each - and the first collective
  completes no earlier than ~100 us (first-call barrier), so every
  collective-dependent Wo block is placed with >=20 us of slack and the
  last Wo blocks are held back as PE filler for the final collective.
* PSUM: proj pool 2 banks (q/k/v/transpose rotate through [128,512] slots),
  scores 2x2 banks, ctx accumulators 2 banks = exactly 8.  The projection
  for tile t+1 and the Wo matmuls are emitted after the attention section
  that hides them; the Tile scheduler slots them into the PE stalls where
  attention waits on the ACT exp stream (exp on ACT is ~1.15 us per
  128-key block vs ~0.65 us of PE work, so without filler the PE idles
  ~40% during attention and the HAM clock gate re-throttles).
* Softmax normalization happens on the receiving core (the a2a payload is
  65 rows per head: 64 unnormalized ctx dims + the denominator row from a
  trailing ones-column in the AV stationary); 16 denominator rows stack on
  the partition axis at the receiver: a 16-lane DVE reciprocal, then 8
  K=16 one-hot selector matmuls broadcast the reciprocals into PSUM in cg
  layout (replacing a DRAM-bounced stride-0 DMA round trip, and nudging
  the PE awake right before the Wo block), then one fused DVE multiply
  normalizes the gathered ctx.  Tail recv chains issue their gather DMAs
  from the Scalar queue (idle in the tail; Sync is not).
* Attention-times-V keeps V plus a trailing ones column as the 65-column
  stationary operand and streams the exp tile; scores use tile_position
  row pairs so the two heads' score matmuls run concurrently; exp is one
  ACT instruction per key block covering both heads; the diagonal tri-mask
  is one DVE multiply per block via a stride-0 broadcast AP over heads.
* A tiny warm-up AllToAll is issued during the load phase so the first real
  collective doesn't pay the ~23 us first-call setup on the critical path.
"""

import sys

sys.path.insert(0, "/opt/trn_rl_repo")

import ml_dtypes
import numpy as np

import concourse.bass as bass
import concourse.tile as tile
from concourse import bacc, mybir
from concourse.bass_utils import run_bass_kernel_spmd

N_CORES = 8
B, S, H, D = 2, 2048, 16, 64
E = H * D            # 1024
T = B * S            # 4096 tokens
DPC = 128            # dims (2 heads) per core
NKC = E // 128       # 8 contraction chunks for the projections
SB = S // 128        # 16 key blocks per batch
PHQ = 512 // N_CORES  # 64 tokens per core per query group
PH = 2 * PHQ         # 128 tokens per core per half-batch
CR = 130             # a2a chunk rows: 2 x (64 ctx dims + den)

F32 = mybir.dt.float32
BF16 = mybir.dt.bfloat16
FP8 = mybir.dt.float8e4
AFT = mybir.ActivationFunctionType
NK2 = 4              # 256-wide contraction chunks for DoubleRow q/k

QT_ORDER = (0, 1, 3, 2)  # hf0 = {0,1} finishes early; q2 (12 blocks) last


def build_program():
    nc = bacc.Bacc("TRN2", target_bir_lowering=False, debug=False,
                   num_devices=N_CORES)

    xT = nc.dram_tensor("xT", [E, T], BF16, kind="ExternalInput").ap()
    # fp8 copies of x and Wq/Wk for DoubleRow q/k projections (v stays
    # bf16: early-token v errors are unprotected by softmax averaging)
    x8T = nc.dram_tensor("x8T", [E, T], FP8, kind="ExternalInput").ap()
    wq8T = nc.dram_tensor("wq8T", [E, DPC], FP8, kind="ExternalInput").ap()
    wk8T = nc.dram_tensor("wk8T", [E, DPC], FP8, kind="ExternalInput").ap()
    wvT = nc.dram_tensor("wvT", [E, DPC], BF16, kind="ExternalInput").ap()
    woT = nc.dram_tensor("woT", [E, E], BF16, kind="ExternalInput").ap()
    bq = nc.dram_tensor("bq", [DPC, 1], F32, kind="ExternalInput").ap()
    bk = nc.dram_tensor("bk", [DPC, 1], F32, kind="ExternalInput").ap()
    bv = nc.dram_tensor("bv", [DPC, 1], F32, kind="ExternalInput").ap()
    bo = nc.dram_tensor("bo", [E], F32, kind="ExternalInput").ap()
    # single 128x128 lower-triangular (k_local <= q_local) mask
    tri = nc.dram_tensor("tri", [128, 128], BF16, kind="ExternalInput").ap()
    ident = nc.dram_tensor("ident", [128, 128], BF16, kind="ExternalInput").ap()
    # selk[k, kc*128+p] = 1 iff k == 2*kc + p//64: one-hot selectors that
    # broadcast the 16 per-(peer,head) reciprocal rows to the cg layout
    # via 8 tiny K=16 matmuls (replaces a DRAM-bounced stride-0 DMA).
    selk = nc.dram_tensor("selk", [16, NKC * 128], BF16,
                          kind="ExternalInput").ap()
    out = nc.dram_tensor("out", [T // N_CORES, E], F32, kind="ExternalOutput").ap()

    with tile.TileContext(nc) as tc:
        with (
            tc.tile_pool(name="consts", bufs=1) as consts,
            tc.tile_pool(name="state", bufs=1) as state,
            tc.tile_pool(name="ep", bufs=6) as ep,
            tc.tile_pool(name="op", bufs=4) as op,
            tc.tile_pool(name="ps_p", bufs=2, space="PSUM") as ps_p,
            tc.tile_pool(name="ps_s", bufs=2, space="PSUM") as ps_s,
            tc.tile_pool(name="ps_c", bufs=2, space="PSUM") as ps_c,
            tc.tile_pool(name="dram", bufs=1, space="DRAM") as dram,
        ):
            # ---- warm-up collective: absorbs the first-AllToAll setup cost
            # while the DMA engines are still loading x ----------------------
            wu_s = consts.tile([128, 16], BF16)
            nc.vector.memset(wu_s[:], 0.0)
            wu_in = dram.tile([N_CORES, 16, 16], BF16, tag="wu_in", name="wu_in")
            wu_out = dram.tile([N_CORES, 16, 16], BF16, tag="wu_out",
                               name="wu_out")
            nc.sync.dma_start(out=wu_in[:], in_=wu_s[:])
            nc.gpsimd.collective_compute(
                "AllToAll",
                mybir.AluOpType.bypass,
                replica_groups=[list(range(N_CORES))],
                ins=[wu_in.opt()],
                outs=[wu_out.opt()],
            )

            def chunked(dram_ap, cols, kc0, kcn):
                # DRAM [E, cols] viewed as [p, kc, cols]: row kc*128+p
                return bass.AP(tensor=dram_ap.tensor,
                               offset=dram_ap.offset + kc0 * 128 * cols,
                               ap=[[cols, 128], [128 * cols, kcn], [1, cols]])

            # ---- loads in deadline order ---------------------------------
            # fp8 weights/x in DoubleRow layout: contraction index
            # k = kc2*256 + i*128 + p -> [p, kc2, i, cols]
            def dr_ap(dram_ap, cols, toff=0, width=None):
                return bass.AP(tensor=dram_ap.tensor,
                               offset=dram_ap.offset + toff,
                               ap=[[cols, 128], [256 * cols, NK2],
                                   [128 * cols, 2], [1, width or cols]])

            wq8_sb = consts.tile([128, NK2, 2, DPC], FP8)
            bq_sb = consts.tile([128, 1], F32)
            nc.sync.dma_start(out=wq8_sb[:], in_=dr_ap(wq8T, DPC))
            nc.sync.dma_start(out=bq_sb[:], in_=bq[:])
            x8_t = [None] * NKC
            x8_t[0] = state.tile([128, NK2, 2, 512], FP8, name="x8_0")
            nc.sync.dma_start(out=x8_t[0][:], in_=dr_ap(x8T, T, 0, 512))
            wk8_sb = consts.tile([128, NK2, 2, DPC], FP8)
            bk_sb = consts.tile([128, 1], F32)
            nc.sync.dma_start(out=wk8_sb[:], in_=dr_ap(wk8T, DPC))
            nc.sync.dma_start(out=bk_sb[:], in_=bk[:])
            x_t = [None] * NKC
            x8_t[1] = state.tile([128, NK2, 2, 512], FP8, name="x8_1")
            nc.sync.dma_start(out=x8_t[1][:], in_=dr_ap(x8T, T, 512, 512))
            x8_t[2] = state.tile([128, NK2, 2, 512], FP8, name="x8_2")
            nc.sync.dma_start(out=x8_t[2][:], in_=dr_ap(x8T, T, 1024, 512))
            # x tile 0 (bf16, for the V projection) in two halves
            x0a = state.tile([128, NKC // 2, 512], BF16, name="x0a")
            x0b = state.tile([128, NKC // 2, 512], BF16, name="x0b")

            def x_ap(tt, kc0, kcn):
                return bass.AP(tensor=xT.tensor,
                               offset=xT.offset + tt * 512 + kc0 * 128 * T,
                               ap=[[T, 128], [128 * T, kcn], [1, 512]])

            nc.sync.dma_start(out=x0a[:], in_=x_ap(0, 0, 4))
            wv_sb = consts.tile([128, NKC, DPC], BF16)
            bv_sb = consts.tile([128, 1], F32)
            nc.sync.dma_start(out=x0b[:], in_=x_ap(0, 4, 4))
            nc.sync.dma_start(out=wv_sb[:], in_=chunked(wvT, DPC, 0, NKC))
            nc.sync.dma_start(out=bv_sb[:], in_=bv[:])
            x_t[1] = state.tile([128, NKC, 512], BF16, name="x1")
            nc.sync.dma_start(out=x_t[1][:], in_=x_ap(1, 0, NKC))
            tri_sb = consts.tile([128, 128], BF16)
            nc.sync.dma_start(out=tri_sb[:], in_=tri[:])
            id_sb = consts.tile([128, 128], BF16)
            nc.sync.dma_start(out=id_sb[:], in_=ident[:])
            x_t[2] = state.tile([128, NKC, 512], BF16, name="x2")
            nc.sync.dma_start(out=x_t[2][:], in_=x_ap(2, 0, NKC))
            for tt in range(3, NKC):
                x8t = state.tile([128, NK2, 2, 512], FP8, name=f"x8_{tt}")
                nc.sync.dma_start(out=x8t[:], in_=dr_ap(x8T, T, tt * 512, 512))
                x8_t[tt] = x8t
                xt = state.tile([128, NKC, 512], BF16, name=f"x{tt}")
                nc.sync.dma_start(out=xt[:], in_=x_ap(tt, 0, NKC))
                x_t[tt] = xt
            wo_sb = consts.tile([128, NKC, E], BF16)
            nc.sync.dma_start(out=wo_sb[:], in_=chunked(woT, E, 0, NKC))
            selk_sb = consts.tile([16, NKC, 128], BF16)
            nc.sync.dma_start(out=selk_sb[:], in_=selk[:].rearrange(
                "k (c p) -> k c p", c=NKC))
            bo_bc = consts.tile([128, E], F32)
            nc.sync.dma_start(
                out=bo_bc[:],
                in_=bass.AP(tensor=bo.tensor, offset=bo.offset,
                            ap=[[0, 128], [1, E]]),
            )

            # ---- persistent activations -----------------------------------
            qT_sb = state.tile([128, T], BF16)   # [2-head dims, tokens]
            kT_sb = state.tile([128, T], BF16)
            vT_sb = state.tile([128, T], BF16)
            # per 128-token block: [64 v-dims, ones] per head -> the AV
            # matmul's 65-column stationary operand; the ones column makes
            # PSUM row 64 the softmax denominator.
            vN_sb = state.tile([128, T // 128, 130], BF16)
            # unnormalized ctx^T + den: rows 0-63 ctx dims, row 64 den
            ctx2_sb = state.tile([65, 2, T], BF16)

            nc.vector.memset(vN_sb[:, :, 64:65], 1.0)
            nc.vector.memset(vN_sb[:, :, 129:130], 1.0)

            # ---- stage builders -------------------------------------------
            def emit_proj_qk(tt):
                # fp8 q/k projections only - gate the score stream on just
                # the (small) fp8 x tiles so attention starts early
                ts = slice(tt * 512, (tt + 1) * 512)
                ps_q = ps_p.tile([128, 512], F32, tag="p", name="ps_q")
                for kc2 in range(NK2):
                    nc.tensor.matmul(ps_q[:], wq8_sb[:, kc2, :, :],
                                     x8_t[tt][:, kc2, :, :],
                                     start=(kc2 == 0), stop=(kc2 == NK2 - 1),
                                     perf_mode=mybir.MatmulPerfMode.DoubleRow,
                                     skip_group_check=True)
                nc.vector.tensor_scalar_add(qT_sb[:, ts], ps_q[:], bq_sb[:])
                ps_k = ps_p.tile([128, 512], F32, tag="p", name="ps_k")
                for kc2 in range(NK2):
                    nc.tensor.matmul(ps_k[:], wk8_sb[:, kc2, :, :],
                                     x8_t[tt][:, kc2, :, :],
                                     start=(kc2 == 0), stop=(kc2 == NK2 - 1),
                                     perf_mode=mybir.MatmulPerfMode.DoubleRow,
                                     skip_group_check=True)
                nc.vector.tensor_scalar_add(kT_sb[:, ts], ps_k[:], bk_sb[:])

            def emit_proj_v(tt):
                # bf16 V projection + transposes, deferred to each tile's
                # first AV deadline (paced by the bf16 x tile loads)
                ts = slice(tt * 512, (tt + 1) * 512)
                xa = (x0a, x0b) if tt == 0 else (x_t[tt],)
                nch = NKC // len(xa)
                ps_v = ps_p.tile([128, 512], F32, tag="p", name="ps_v")
                for kc in range(NKC):
                    xt = xa[kc // nch]
                    nc.tensor.matmul(ps_v[:], wv_sb[:, kc, :],
                                     xt[:, kc % nch, :],
                                     start=(kc == 0), stop=(kc == NKC - 1),
                                     skip_group_check=True)
                nc.vector.tensor_scalar_add(vT_sb[:, ts], ps_v[:], bv_sb[:])
                tp_ps = ps_p.tile([128, 4, 128], BF16, tag="p", name="tp_ps")
                for ti, tb in enumerate(range(tt * 4, tt * 4 + 4)):
                    nc.tensor.transpose(
                        tp_ps[:, ti, :], vT_sb[:, tb * 128:(tb + 1) * 128],
                        id_sb[:])
                    nc.vector.tensor_copy(vN_sb[:, tb, 0:64],
                                          tp_ps[:, ti, 0:64])
                    nc.vector.tensor_copy(vN_sb[:, tb, 65:129],
                                          tp_ps[:, ti, 64:128])

            def emit_proj(tt):
                emit_proj_qk(tt)
                emit_proj_v(tt)

            def emit_attn(b, qt):
                t0 = b * S
                q0 = t0 + qt * 512
                nkb = 4 * qt + 4

                def emit_scores(kb):
                    c0 = max(kb - 4 * qt, 0) * 128
                    s = ps_s.tile([128, 2, 512], F32, tag="s", name="s_ps")
                    for h in range(2):
                        d0 = h * 64
                        nc.tensor.matmul(
                            s[:, h, c0:512],
                            kT_sb[d0:d0 + 64,
                                  t0 + kb * 128:t0 + (kb + 1) * 128],
                            qT_sb[d0:d0 + 64, q0 + c0:q0 + 512],
                            start=True, stop=True, skip_group_check=True)
                    return s

                s_tiles = {0: emit_scores(0)}
                cn = [ps_c.tile([128, 512], F32, tag="cn", name=f"cn{h}")
                      for h in range(2)]
                for kb in range(nkb):
                    m = kb - 4 * qt
                    c0 = max(m, 0) * 128
                    if kb + 1 < nkb:
                        s_tiles[kb + 1] = emit_scores(kb + 1)
                    s = s_tiles.pop(kb)
                    e = ep.tile([128, 2, 512], BF16, tag="e", name="e_sb")
                    # Wq and Wk are pre-scaled by 64 on the host (their
                    # 0.02-sigma values would be subnormal in fp8e4m3), so
                    # the logits carry an extra 64*64 factor undone here.
                    nc.scalar.activation(e[:, :, c0:512], s[:, :, c0:512],
                                         AFT.Exp, scale=0.125 / 4096.0)
                    if m >= 0:  # triangular block on the diagonal
                        nc.vector.tensor_mul(
                            e[:, :, c0:c0 + 128], e[:, :, c0:c0 + 128],
                            tri_sb[:].unsqueeze(1).broadcast_to((128, 2, 128)))
                    for h in range(2):
                        nc.tensor.matmul(
                            cn[h][0:65, c0:512],
                            vN_sb[:, b * SB + kb, 65 * h:65 * h + 65],
                            e[:, h, c0:512],
                            start=(kb == 0), stop=(kb == nkb - 1),
                            skip_group_check=True)

                # stage unnormalized ctx + den rows for the AllToAll
                for h in range(2):
                    nc.vector.tensor_copy(ctx2_sb[:, h, q0:q0 + 512],
                                          cn[h][0:65, :])

            def emit_half_a2a(b, hf):
                # half-batch = query groups {2hf, 2hf+1}; peer j gets this
                # core's heads' ctx for j's 128 half-batch tokens.
                base = b * S + hf * (S // 2)
                ctxd = dram.tile([N_CORES, CR, PH], BF16, tag="ctxd",
                                 name="ctxd", bufs=4)
                for h in range(2):
                    nc.sync.dma_start(
                        out=bass.AP(tensor=ctxd.tensor,
                                    offset=ctxd[0].offset + h * 65 * PH,
                                    ap=[[PH, 65], [CR * PH, N_CORES],
                                        [1, PH]]),
                        in_=ctx2_sb[:, h, base:base + S // 2].rearrange(
                            "p (j t) -> p j t", j=N_CORES))
                recv = dram.tile([N_CORES, CR, PH], BF16, tag="recv",
                                 name="recv", bufs=4)
                nc.gpsimd.collective_compute(
                    "AllToAll",
                    mybir.AluOpType.bypass,
                    replica_groups=[list(range(N_CORES))],
                    ins=[ctxd.opt()],
                    outs=[recv.opt()],
                )
                return recv

            def emit_half_recv(b, hf, recv, eng=None):
                # gather + normalize the received ctx for one half-batch.
                # eng picks the DMA issue queue: Sync mid-kernel, Scalar for
                # the tail chains (ACT is idle there, Sync is not).
                eng = eng or nc.sync
                cg_sb = op.tile([128, NKC, PH], BF16, tag="cg_sb", name="cg_sb",
                                bufs=2)
                den16 = op.tile([16, PH], BF16, tag="den16", name="den16",
                                bufs=2)
                r0 = recv[0]
                # den16 gathers first: the reciprocal heads the broadcast
                # chain, so it starts while cg still transfers
                eng.dma_start(
                    out=den16[:],
                    in_=bass.AP(tensor=r0.tensor,
                                offset=r0.offset + 64 * PH,
                                ap=[[CR * PH, N_CORES], [65 * PH, 2],
                                    [1, PH]]))
                for h in range(2):
                    eng.dma_start(
                        out=cg_sb[h * 64:(h + 1) * 64, :, :],
                        in_=bass.AP(tensor=r0.tensor,
                                    offset=r0.offset + h * 65 * PH,
                                    ap=[[PH, 64], [CR * PH, N_CORES],
                                        [1, PH]]))
                r16 = op.tile([16, PH], BF16, tag="r16", name="r16", bufs=2)
                with nc.allow_low_precision(
                        reason="bf16 softmax scale, |rel err| ~4e-3 ok"):
                    nc.vector.reciprocal(r16[:], den16[:])
                # broadcast r16 to the cg layout with 8 K=16 matmuls:
                # rmap_ps[p, kc, t] = r16[2*kc + p//64, t]; also nudges the
                # PE awake right before the Wo block that follows.
                rmap_ps = ps_s.tile([128, 2, 512], F32, tag="s",
                                    name="rmap_ps")
                for kc in range(NKC):
                    nc.tensor.matmul(
                        rmap_ps[:, kc >> 2,
                                (kc & 3) * 128:((kc & 3) + 1) * 128],
                        selk_sb[:, kc, :], r16[:],
                        start=True, stop=True, skip_group_check=True)
                nc.vector.tensor_mul(
                    cg_sb[:].rearrange("p a b -> p (a b)"),
                    cg_sb[:].rearrange("p a b -> p (a b)"),
                    rmap_ps[:].rearrange("p a b -> p (a b)"))
                return b, hf, cg_sb

            def emit_half_wo(b, hf, cg_sb):
                o_sb = op.tile([PH, E], F32, tag="o_sb", name="o_sb", bufs=3)
                for et in range(2):
                    ps = ps_s.tile([128, 2, 512], F32, tag="s", name="c_ps")
                    for kc in range(NKC):
                        nc.tensor.matmul(
                            ps[0:PH, 0, :],
                            cg_sb[:, kc, :],
                            wo_sb[:, kc, et * 512:(et + 1) * 512],
                            start=(kc == 0), stop=(kc == NKC - 1),
                            skip_group_check=True)
                    nc.vector.tensor_add(
                        o_sb[:, et * 512:(et + 1) * 512], ps[0:PH, 0, :],
                        bo_bc[0:PH, et * 512:(et + 1) * 512])
                    r0w = (b * 2 + hf) * PH
                    nc.sync.dma_start(
                        out=out[r0w:r0w + PH, et * 512:(et + 1) * 512],
                        in_=o_sb[:, et * 512:(et + 1) * 512])

            # ---- schedule -------------------------------------------------
            # qt order {2,3,0,1} per batch: half 1 (28 key-blocks of work)
            # finishes first and its a2a fires ~70 us before the end; half 0
            # ({q0,q1}, only 12 key-blocks) finishes last so the FINAL
            # collective fires ~15 us of attention + ~15 us of held-back Wo
            # work before the PE drains.  4 collectives, spaced >=15 us
            # (back-to-back a2as degrade ~3x on this part) and the first
            # completes no earlier than ~100 us (first-call barrier), so
            # every collective-dependent Wo block is placed with >=20 us of
            # slack after its gating collective's expected completion.
            emit_proj_qk(0)
            emit_proj_qk(1)
            emit_proj_qk(2)
            emit_proj_v(0)
            # -------- batch 0
            emit_attn(0, 2)
            emit_proj_v(1)
            emit_proj_v(2)
            emit_proj(3)
            emit_attn(0, 3)
            rB = emit_half_a2a(0, 1)          # cc1, trigger ~55us
            emit_attn(0, 0)
            emit_proj(4)
            emit_attn(0, 1)
            rA = emit_half_a2a(0, 0)          # cc2, trigger ~75us
            emit_proj(5)
            emit_proj(6)
            # -------- batch 1
            emit_attn(1, 2)
            emit_proj(7)
            emit_attn(1, 3)
            rD = emit_half_a2a(1, 1)          # cc3, trigger ~125us
            # recv chains emit AFTER the a2a staging so their cc-gated
            # gather DMAs can never delay staging on the in-order Sync queue
            args01 = emit_half_recv(0, 1, rB)  # cc1 done ~105us
            emit_half_wo(*args01)             # PE ~130us
            emit_attn(1, 0)
            emit_attn(1, 1)
            rC = emit_half_a2a(1, 0)          # cc4 (tail), trigger ~142us
            # tail: independent Wo blocks keep the PE busy through the
            # final collective + its recv chain
            args00 = emit_half_recv(0, 0, rA)  # cc2 done ~110us
            emit_half_wo(*args00)
            args11 = emit_half_recv(1, 1, rD, eng=nc.scalar)  # cc3 ~137us
            emit_half_wo(*args11)
            args10 = emit_half_recv(1, 0, rC, eng=nc.scalar)
            emit_half_wo(*args10)

    nc.compile()
    return nc


_NC = None


def _get_program():
    global _NC
    if _NC is None:
        _NC = build_program()
    return _NC


def _bf(a):
    return np.ascontiguousarray(a).astype(ml_dtypes.bfloat16)


def _f8(a):
    return np.ascontiguousarray(a).astype(ml_dtypes.float8_e4m3)


def kernel(x, Wq, bq, Wk, bk, Wv, bv, Wo, bo, _trace=False, _trace_kwargs=None):
    x = np.asarray(x, np.float32)
    Wq, Wk, Wv, Wo = (np.asarray(w, np.float32) for w in (Wq, Wk, Wv, Wo))
    bq, bk, bv, bo = (np.asarray(v, np.float32) for v in (bq, bk, bv, bo))

    xf = x.reshape(T, E).T
    xT = _bf(xf)
    x8T = _f8(xf)
    i = np.arange(128)
    tri = _bf((i[:, None] <= i[None, :]).astype(np.float32))
    ident = _bf(np.eye(128, dtype=np.float32))
    k16 = np.arange(16)[:, None]
    kcp = (2 * (np.arange(NKC * 128) // 128) + (np.arange(NKC * 128) % 128) // 64)
    selk = _bf((k16 == kcp[None, :]).astype(np.float32))

    in_maps = []
    for c in range(N_CORES):
        sl = slice(c * DPC, (c + 1) * DPC)
        in_maps.append({
            "xT": xT,
            "x8T": x8T,
            "wq8T": _f8(64.0 * Wq[sl, :].T),
            "wk8T": _f8(64.0 * Wk[sl, :].T),
            "wvT": _bf(Wv[sl, :].T),
            "woT": _bf(Wo.T),
            "bq": 64.0 * bq[sl].reshape(DPC, 1),
            "bk": 64.0 * bk[sl].reshape(DPC, 1),
            "bv": bv[sl].reshape(DPC, 1).copy(),
            "bo": bo,
            "tri": tri,
            "ident": ident,
            "selk": selk,
        })

    nc = _get_program()
    res = run_bass_kernel_spmd(nc, in_maps, list(range(N_CORES)),
                               trace=_trace, **(_trace_kwargs or {}))
    # out[c] rows are [batch, half, 128]: row (b, hf, r) holds global
    # token b*2048 + hf*1024 + c*128 + r.
    stacked = np.stack([res.results[i]["out"].reshape(B, 2, PH, E)
                        for i in range(N_CORES)], axis=2)
    full = stacked.reshape(T, E)
    if _trace:
        return full.reshape(B, S, E), res
    return full.reshape(B, S, E)


# revision 50
# speedup vs baseline: 1.1359x; 1.0339x over previous
"""Multi-head attention (B=2, S=2048, H=16, D=64) on 8 Trainium2 NeuronCores.

Head-parallel tensor parallelism: core c owns heads {2c, 2c+1} (a 128-dim
slice of the model dim): column-parallel QKV projections and local causal
attention for its 2 heads, then an AllToAll of bf16 context vectors (one
512-token query group at a time) and a full-width Wo projection for this
core's own disjoint 64-token output slices.

Schedule (v6), shaped by trace measurements:

* q/k projections run in fp8e4m3 with perf_mode=DoubleRow (256-wide
  contraction chunks, ~1.5x PE throughput); Wq/Wk are pre-scaled by 64 on
  the host (their 0.02-sigma entries would land in e4m3's subnormal range)
  and the 64*64 logit factor is undone inside the exp's free scale.  The
  V projection stays bf16: early-token v errors are unprotected by
  softmax averaging and would breach the error budget.
* Startup: wq8/bq and the fp8 x tile 0 are the first DMAs issued;
  everything else loads behind them in deadline order (tri before the
  first diagonal block, wo/bo last).
* Query groups run in order {2, 3, 0, 1} per batch so the LAST collective
  (half 0, only 12 key-blocks of attention) fires well before the PE
  drains.  4 collectives spaced >=15 us: per-qt granularity (8
  collectives) was measured to DEGRADE - back-to-back AllToAlls on this
  part grow from ~6 us to ~22 us each - and the first collective
  completes no earlier than ~100 us (first-call barrier), so every
  collective-dependent Wo block is placed with >=20 us of slack and the
  last Wo blocks are held back as PE filler for the final collective.
* PSUM: proj pool 2 banks (q/k/v/transpose rotate through [128,512] slots),
  scores 2x2 banks, ctx accumulators 2 banks = exactly 8.  The projection
  for tile t+1 and the Wo matmuls are emitted after the attention section
  that hides them; the Tile scheduler slots them into the PE stalls where
  attention waits on the ACT exp stream (exp on ACT is ~1.15 us per
  128-key block vs ~0.65 us of PE work, so without filler the PE idles
  ~40% during attention and the HAM clock gate re-throttles).
* Softmax normalization happens on the receiving core (the a2a payload is
  65 rows per head: 64 unnormalized ctx dims + the denominator row from a
  trailing ones-column in the AV stationary); 16 denominator rows stack on
  the partition axis at the receiver: a 16-lane DVE reciprocal, then 8
  K=16 one-hot selector matmuls broadcast the reciprocals into PSUM in cg
  layout (replacing a DRAM-bounced stride-0 DMA round trip, and nudging
  the PE awake right before the Wo block), then one fused DVE multiply
  normalizes the gathered ctx.  Tail recv chains issue their gather DMAs
  from the Scalar queue (idle in the tail; Sync is not).
* Attention-times-V keeps V plus a trailing ones column as the 65-column
  stationary operand and streams the exp tile; scores use tile_position
  row pairs so the two heads' score matmuls run concurrently; exp is one
  ACT instruction per key block covering both heads; the diagonal tri-mask
  is one DVE multiply per block via a stride-0 broadcast AP over heads.
* A tiny warm-up AllToAll is issued during the load phase so the first real
  collective doesn't pay the ~23 us first-call setup on the critical path.
"""

import sys

sys.path.insert(0, "/opt/trn_rl_repo")

import ml_dtypes
import numpy as np

import concourse.bass as bass
import concourse.tile as tile
from concourse import bacc, mybir
from concourse.bass_utils import run_bass_kernel_spmd

N_CORES = 8
B, S, H, D = 2, 2048, 16, 64
E = H * D            # 1024
T = B * S            # 4096 tokens
DPC = 128            # dims (2 heads) per core
NKC = E // 128       # 8 contraction chunks for the projections
SB = S // 128        # 16 key blocks per batch
PHQ = 512 // N_CORES  # 64 tokens per core per query group
PH = 2 * PHQ         # 128 tokens per core per half-batch
CR = 130             # a2a chunk rows: 2 x (64 ctx dims + den)

F32 = mybir.dt.float32
BF16 = mybir.dt.bfloat16
FP8 = mybir.dt.float8e4
AFT = mybir.ActivationFunctionType
NK2 = 4              # 256-wide contraction chunks for DoubleRow q/k

QT_ORDER = (0, 1, 3, 2)  # hf0 = {0,1} finishes early; q2 (12 blocks) last


def build_program():
    nc = bacc.Bacc("TRN2", target_bir_lowering=False, debug=False,
                   num_devices=N_CORES)

    xT = nc.dram_tensor("xT", [E, T], BF16, kind="ExternalInput").ap()
    # fp8 copies of x and Wq/Wk for DoubleRow q/k projections (v stays
    # bf16: early-token v errors are unprotected by softmax averaging)
    x8T = nc.dram_tensor("x8T", [E, T], FP8, kind="ExternalInput").ap()
    wq8T = nc.dram_tensor("wq8T", [E, DPC], FP8, kind="ExternalInput").ap()
    wk8T = nc.dram_tensor("wk8T", [E, DPC], FP8, kind="ExternalInput").ap()
    wvT = nc.dram_tensor("wvT", [E, DPC], BF16, kind="ExternalInput").ap()
    woT = nc.dram_tensor("woT", [E, E], BF16, kind="ExternalInput").ap()
    bq = nc.dram_tensor("bq", [DPC, 1], F32, kind="ExternalInput").ap()
    bk = nc.dram_tensor("bk", [DPC, 1], F32, kind="ExternalInput").ap()
    bv = nc.dram_tensor("bv", [DPC, 1], F32, kind="ExternalInput").ap()
    bo = nc.dram_tensor("bo", [E], F32, kind="ExternalInput").ap()
    # single 128x128 lower-triangular (k_local <= q_local) mask
    tri = nc.dram_tensor("tri", [128, 128], BF16, kind="ExternalInput").ap()
    ident = nc.dram_tensor("ident", [128, 128], BF16, kind="ExternalInput").ap()
    # selk[k, kc*128+p] = 1 iff k == 2*kc + p//64: one-hot selectors that
    # broadcast the 16 per-(peer,head) reciprocal rows to the cg layout
    # via 8 tiny K=16 matmuls (replaces a DRAM-bounced stride-0 DMA).
    selk = nc.dram_tensor("selk", [16, NKC * 128], BF16,
                          kind="ExternalInput").ap()
    out = nc.dram_tensor("out", [T // N_CORES, E], F32, kind="ExternalOutput").ap()

    with tile.TileContext(nc) as tc:
        with (
            tc.tile_pool(name="consts", bufs=1) as consts,
            tc.tile_pool(name="state", bufs=1) as state,
            tc.tile_pool(name="ep", bufs=6) as ep,
            tc.tile_pool(name="op", bufs=4) as op,
            tc.tile_pool(name="ps_p", bufs=2, space="PSUM") as ps_p,
            tc.tile_pool(name="ps_s", bufs=2, space="PSUM") as ps_s,
            tc.tile_pool(name="ps_c", bufs=2, space="PSUM") as ps_c,
            tc.tile_pool(name="dram", bufs=1, space="DRAM") as dram,
        ):
            # ---- warm-up collective: absorbs the first-AllToAll setup cost
            # while the DMA engines are still loading x ----------------------
            wu_s = consts.tile([128, 16], BF16)
            nc.vector.memset(wu_s[:], 0.0)
            wu_in = dram.tile([N_CORES, 16, 16], BF16, tag="wu_in", name="wu_in")
            wu_out = dram.tile([N_CORES, 16, 16], BF16, tag="wu_out",
                               name="wu_out")
            nc.sync.dma_start(out=wu_in[:], in_=wu_s[:])
            nc.gpsimd.collective_compute(
                "AllToAll",
                mybir.AluOpType.bypass,
                replica_groups=[list(range(N_CORES))],
                ins=[wu_in.opt()],
                outs=[wu_out.opt()],
            )

            def chunked(dram_ap, cols, kc0, kcn):
                # DRAM [E, cols] viewed as [p, kc, cols]: row kc*128+p
                return bass.AP(tensor=dram_ap.tensor,
                               offset=dram_ap.offset + kc0 * 128 * cols,
                               ap=[[cols, 128], [128 * cols, kcn], [1, cols]])

            # ---- loads in deadline order ---------------------------------
            # fp8 weights/x in DoubleRow layout: contraction index
            # k = kc2*256 + i*128 + p -> [p, kc2, i, cols]
            def dr_ap(dram_ap, cols, toff=0, width=None):
                return bass.AP(tensor=dram_ap.tensor,
                               offset=dram_ap.offset + toff,
                               ap=[[cols, 128], [256 * cols, NK2],
                                   [128 * cols, 2], [1, width or cols]])

            wq8_sb = consts.tile([128, NK2, 2, DPC], FP8)
            bq_sb = consts.tile([128, 1], F32)
            nc.sync.dma_start(out=wq8_sb[:], in_=dr_ap(wq8T, DPC))
            nc.sync.dma_start(out=bq_sb[:], in_=bq[:])
            # fp8 x tile 0 in two halves so the very first matmuls start
            # after ~0.4 MB of transfers
            x8_t = [None] * NKC
            x8_0a = state.tile([128, NK2 // 2, 2, 512], FP8, name="x8_0a")
            x8_0b = state.tile([128, NK2 // 2, 2, 512], FP8, name="x8_0b")

            def dr_half_ap(kc2_0):
                return bass.AP(tensor=x8T.tensor,
                               offset=x8T.offset + kc2_0 * 256 * T,
                               ap=[[T, 128], [256 * T, NK2 // 2],
                                   [128 * T, 2], [1, 512]])

            nc.sync.dma_start(out=x8_0a[:], in_=dr_half_ap(0))
            nc.sync.dma_start(out=x8_0b[:], in_=dr_half_ap(2))
            wk8_sb = consts.tile([128, NK2, 2, DPC], FP8)
            bk_sb = consts.tile([128, 1], F32)
            nc.sync.dma_start(out=wk8_sb[:], in_=dr_ap(wk8T, DPC))
            nc.sync.dma_start(out=bk_sb[:], in_=bk[:])
            x_t = [None] * NKC
            x8_t[1] = state.tile([128, NK2, 2, 512], FP8, name="x8_1")
            nc.sync.dma_start(out=x8_t[1][:], in_=dr_ap(x8T, T, 512, 512))
            x8_t[2] = state.tile([128, NK2, 2, 512], FP8, name="x8_2")
            nc.sync.dma_start(out=x8_t[2][:], in_=dr_ap(x8T, T, 1024, 512))
            # x tile 0 (bf16, for the V projection) in two halves
            x0a = state.tile([128, NKC // 2, 512], BF16, name="x0a")
            x0b = state.tile([128, NKC // 2, 512], BF16, name="x0b")

            def x_ap(tt, kc0, kcn):
                return bass.AP(tensor=xT.tensor,
                               offset=xT.offset + tt * 512 + kc0 * 128 * T,
                               ap=[[T, 128], [128 * T, kcn], [1, 512]])

            nc.sync.dma_start(out=x0a[:], in_=x_ap(0, 0, 4))
            wv_sb = consts.tile([128, NKC, DPC], BF16)
            bv_sb = consts.tile([128, 1], F32)
            nc.sync.dma_start(out=x0b[:], in_=x_ap(0, 4, 4))
            nc.sync.dma_start(out=wv_sb[:], in_=chunked(wvT, DPC, 0, NKC))
            nc.sync.dma_start(out=bv_sb[:], in_=bv[:])
            x_t[1] = state.tile([128, NKC, 512], BF16, name="x1")
            nc.sync.dma_start(out=x_t[1][:], in_=x_ap(1, 0, NKC))
            tri_sb = consts.tile([128, 128], BF16)
            nc.sync.dma_start(out=tri_sb[:], in_=tri[:])
            id_sb = consts.tile([128, 128], BF16)
            nc.sync.dma_start(out=id_sb[:], in_=ident[:])
            x_t[2] = state.tile([128, NKC, 512], BF16, name="x2")
            nc.sync.dma_start(out=x_t[2][:], in_=x_ap(2, 0, NKC))
            for tt in range(3, NKC):
                x8t = state.tile([128, NK2, 2, 512], FP8, name=f"x8_{tt}")
                nc.sync.dma_start(out=x8t[:], in_=dr_ap(x8T, T, tt * 512, 512))
                x8_t[tt] = x8t
                xt = state.tile([128, NKC, 512], BF16, name=f"x{tt}")
                nc.sync.dma_start(out=xt[:], in_=x_ap(tt, 0, NKC))
                x_t[tt] = xt
            wo_sb = consts.tile([128, NKC, E], BF16)
            nc.sync.dma_start(out=wo_sb[:], in_=chunked(woT, E, 0, NKC))
            selk_sb = consts.tile([16, NKC, 128], BF16)
            nc.sync.dma_start(out=selk_sb[:], in_=selk[:].rearrange(
                "k (c p) -> k c p", c=NKC))
            bo_bc = consts.tile([128, E], F32)
            nc.sync.dma_start(
                out=bo_bc[:],
                in_=bass.AP(tensor=bo.tensor, offset=bo.offset,
                            ap=[[0, 128], [1, E]]),
            )

            # ---- persistent activations -----------------------------------
            qT_sb = state.tile([128, T], BF16)   # [2-head dims, tokens]
            kT_sb = state.tile([128, T], BF16)
            vT_sb = state.tile([128, T], BF16)
            # per 128-token block: [64 v-dims, ones] per head -> the AV
            # matmul's 65-column stationary operand; the ones column makes
            # PSUM row 64 the softmax denominator.
            vN_sb = state.tile([128, T // 128, 130], BF16)
            # unnormalized ctx^T + den: rows 0-63 ctx dims, row 64 den
            ctx2_sb = state.tile([65, 2, T], BF16)

            nc.vector.memset(vN_sb[:, :, 64:65], 1.0)
            nc.vector.memset(vN_sb[:, :, 129:130], 1.0)

            # ---- stage builders -------------------------------------------
            def x8c(tt, kc2):
                if tt == 0:
                    return (x8_0a if kc2 < 2 else x8_0b)[:, kc2 % 2, :, :]
                return x8_t[tt][:, kc2, :, :]

            def emit_proj_qk(tt):
                # fp8 q/k projections only - gate the score stream on just
                # the (small) fp8 x tiles so attention starts early
                ts = slice(tt * 512, (tt + 1) * 512)
                ps_q = ps_p.tile([128, 512], F32, tag="p", name="ps_q")
                for kc2 in range(NK2):
                    nc.tensor.matmul(ps_q[:], wq8_sb[:, kc2, :, :],
                                     x8c(tt, kc2),
                                     start=(kc2 == 0), stop=(kc2 == NK2 - 1),
                                     perf_mode=mybir.MatmulPerfMode.DoubleRow,
                                     skip_group_check=True)
                nc.vector.tensor_scalar_add(qT_sb[:, ts], ps_q[:], bq_sb[:])
                ps_k = ps_p.tile([128, 512], F32, tag="p", name="ps_k")
                for kc2 in range(NK2):
                    nc.tensor.matmul(ps_k[:], wk8_sb[:, kc2, :, :],
                                     x8c(tt, kc2),
                                     start=(kc2 == 0), stop=(kc2 == NK2 - 1),
                                     perf_mode=mybir.MatmulPerfMode.DoubleRow,
                                     skip_group_check=True)
                nc.vector.tensor_scalar_add(kT_sb[:, ts], ps_k[:], bk_sb[:])

            def emit_proj_v(tt):
                # bf16 V projection + transposes, deferred to each tile's
                # first AV deadline (paced by the bf16 x tile loads)
                ts = slice(tt * 512, (tt + 1) * 512)
                xa = (x0a, x0b) if tt == 0 else (x_t[tt],)
                nch = NKC // len(xa)
                ps_v = ps_p.tile([128, 512], F32, tag="p", name="ps_v")
                for kc in range(NKC):
                    xt = xa[kc // nch]
                    nc.tensor.matmul(ps_v[:], wv_sb[:, kc, :],
                                     xt[:, kc % nch, :],
                                     start=(kc == 0), stop=(kc == NKC - 1),
                                     skip_group_check=True)
                nc.vector.tensor_scalar_add(vT_sb[:, ts], ps_v[:], bv_sb[:])
                tp_ps = ps_p.tile([128, 4, 128], BF16, tag="p", name="tp_ps")
                for ti, tb in enumerate(range(tt * 4, tt * 4 + 4)):
                    nc.tensor.transpose(
                        tp_ps[:, ti, :], vT_sb[:, tb * 128:(tb + 1) * 128],
                        id_sb[:])
                    nc.vector.tensor_copy(vN_sb[:, tb, 0:64],
                                          tp_ps[:, ti, 0:64])
                    nc.vector.tensor_copy(vN_sb[:, tb, 65:129],
                                          tp_ps[:, ti, 64:128])

            def emit_proj(tt):
                emit_proj_qk(tt)
                emit_proj_v(tt)

            def emit_attn(b, qt):
                t0 = b * S
                q0 = t0 + qt * 512
                nkb = 4 * qt + 4

                def emit_scores(kb):
                    c0 = max(kb - 4 * qt, 0) * 128
                    s = ps_s.tile([128, 2, 512], F32, tag="s", name="s_ps")
                    for h in range(2):
                        d0 = h * 64
                        nc.tensor.matmul(
                            s[:, h, c0:512],
                            kT_sb[d0:d0 + 64,
                                  t0 + kb * 128:t0 + (kb + 1) * 128],
                            qT_sb[d0:d0 + 64, q0 + c0:q0 + 512],
                            start=True, stop=True, skip_group_check=True)
                    return s

                s_tiles = {0: emit_scores(0)}
                cn = [ps_c.tile([128, 512], F32, tag="cn", name=f"cn{h}")
                      for h in range(2)]
                for kb in range(nkb):
                    m = kb - 4 * qt
                    c0 = max(m, 0) * 128
                    if kb + 1 < nkb:
                        s_tiles[kb + 1] = emit_scores(kb + 1)
                    s = s_tiles.pop(kb)
                    e = ep.tile([128, 2, 512], BF16, tag="e", name="e_sb")
                    # Wq and Wk are pre-scaled by 64 on the host (their
                    # 0.02-sigma values would be subnormal in fp8e4m3), so
                    # the logits carry an extra 64*64 factor undone here.
                    nc.scalar.activation(e[:, :, c0:512], s[:, :, c0:512],
                                         AFT.Exp, scale=0.125 / 4096.0)
                    if m >= 0:  # triangular block on the diagonal
                        nc.vector.tensor_mul(
                            e[:, :, c0:c0 + 128], e[:, :, c0:c0 + 128],
                            tri_sb[:].unsqueeze(1).broadcast_to((128, 2, 128)))
                    for h in range(2):
                        nc.tensor.matmul(
                            cn[h][0:65, c0:512],
                            vN_sb[:, b * SB + kb, 65 * h:65 * h + 65],
                            e[:, h, c0:512],
                            start=(kb == 0), stop=(kb == nkb - 1),
                            skip_group_check=True)

                # stage unnormalized ctx + den rows for the AllToAll
                for h in range(2):
                    nc.vector.tensor_copy(ctx2_sb[:, h, q0:q0 + 512],
                                          cn[h][0:65, :])

            def emit_half_a2a(b, hf):
                # half-batch = query groups {2hf, 2hf+1}; peer j gets this
                # core's heads' ctx for j's 128 half-batch tokens.
                base = b * S + hf * (S // 2)
                ctxd = dram.tile([N_CORES, CR, PH], BF16, tag="ctxd",
                                 name="ctxd", bufs=4)
                for h in range(2):
                    nc.sync.dma_start(
                        out=bass.AP(tensor=ctxd.tensor,
                                    offset=ctxd[0].offset + h * 65 * PH,
                                    ap=[[PH, 65], [CR * PH, N_CORES],
                                        [1, PH]]),
                        in_=ctx2_sb[:, h, base:base + S // 2].rearrange(
                            "p (j t) -> p j t", j=N_CORES))
                recv = dram.tile([N_CORES, CR, PH], BF16, tag="recv",
                                 name="recv", bufs=4)
                nc.gpsimd.collective_compute(
                    "AllToAll",
                    mybir.AluOpType.bypass,
                    replica_groups=[list(range(N_CORES))],
                    ins=[ctxd.opt()],
                    outs=[recv.opt()],
                )
                return recv

            def emit_half_recv(b, hf, recv, eng=None):
                # gather + normalize the received ctx for one half-batch.
                # eng picks the DMA issue queue: Sync mid-kernel, Scalar for
                # the tail chains (ACT is idle there, Sync is not).
                eng = eng or nc.sync
                cg_sb = op.tile([128, NKC, PH], BF16, tag="cg_sb", name="cg_sb",
                                bufs=2)
                den16 = op.tile([16, PH], BF16, tag="den16", name="den16",
                                bufs=2)
                r0 = recv[0]
                # den16 gathers first: the reciprocal heads the broadcast
                # chain, so it starts while cg still transfers
                eng.dma_start(
                    out=den16[:],
                    in_=bass.AP(tensor=r0.tensor,
                                offset=r0.offset + 64 * PH,
                                ap=[[CR * PH, N_CORES], [65 * PH, 2],
                                    [1, PH]]))
                for h in range(2):
                    eng.dma_start(
                        out=cg_sb[h * 64:(h + 1) * 64, :, :],
                        in_=bass.AP(tensor=r0.tensor,
                                    offset=r0.offset + h * 65 * PH,
                                    ap=[[PH, 64], [CR * PH, N_CORES],
                                        [1, PH]]))
                r16 = op.tile([16, PH], BF16, tag="r16", name="r16", bufs=2)
                with nc.allow_low_precision(
                        reason="bf16 softmax scale, |rel err| ~4e-3 ok"):
                    nc.vector.reciprocal(r16[:], den16[:])
                # broadcast r16 to the cg layout with 8 K=16 matmuls:
                # rmap_ps[p, kc, t] = r16[2*kc + p//64, t]; also nudges the
                # PE awake right before the Wo block that follows.
                rmap_ps = ps_s.tile([128, 2, 512], F32, tag="s",
                                    name="rmap_ps")
                for kc in range(NKC):
                    nc.tensor.matmul(
                        rmap_ps[:, kc >> 2,
                                (kc & 3) * 128:((kc & 3) + 1) * 128],
                        selk_sb[:, kc, :], r16[:],
                        start=True, stop=True, skip_group_check=True)
                nc.vector.tensor_mul(
                    cg_sb[:].rearrange("p a b -> p (a b)"),
                    cg_sb[:].rearrange("p a b -> p (a b)"),
                    rmap_ps[:].rearrange("p a b -> p (a b)"))
                return b, hf, cg_sb

            def emit_half_wo(b, hf, cg_sb):
                o_sb = op.tile([PH, E], F32, tag="o_sb", name="o_sb", bufs=3)
                for et in range(2):
                    ps = ps_s.tile([128, 2, 512], F32, tag="s", name="c_ps")
                    for kc in range(NKC):
                        nc.tensor.matmul(
                            ps[0:PH, 0, :],
                            cg_sb[:, kc, :],
                            wo_sb[:, kc, et * 512:(et + 1) * 512],
                            start=(kc == 0), stop=(kc == NKC - 1),
                            skip_group_check=True)
                    nc.vector.tensor_add(
                        o_sb[:, et * 512:(et + 1) * 512], ps[0:PH, 0, :],
                        bo_bc[0:PH, et * 512:(et + 1) * 512])
                    r0w = (b * 2 + hf) * PH
                    nc.sync.dma_start(
                        out=out[r0w:r0w + PH, et * 512:(et + 1) * 512],
                        in_=o_sb[:, et * 512:(et + 1) * 512])

            # ---- schedule -------------------------------------------------
            # qt order {2,3,0,1} per batch: half 1 (28 key-blocks of work)
            # finishes first and its a2a fires ~70 us before the end; half 0
            # ({q0,q1}, only 12 key-blocks) finishes last so the FINAL
            # collective fires ~15 us of attention + ~15 us of held-back Wo
            # work before the PE drains.  4 collectives, spaced >=15 us
            # (back-to-back a2as degrade ~3x on this part) and the first
            # completes no earlier than ~100 us (first-call barrier), so
            # every collective-dependent Wo block is placed with >=20 us of
            # slack after its gating collective's expected completion.
            emit_proj_qk(0)
            emit_proj_qk(1)
            emit_proj_qk(2)
            emit_proj_v(0)
            # -------- batch 0
            emit_attn(0, 2)
            emit_proj_v(1)
            emit_proj_v(2)
            emit_proj(3)
            emit_attn(0, 3)
            rB = emit_half_a2a(0, 1)          # cc1, trigger ~55us
            emit_attn(0, 0)
            emit_proj(4)
            emit_attn(0, 1)
            rA = emit_half_a2a(0, 0)          # cc2, trigger ~75us
            emit_proj(5)
            emit_proj(6)
            # -------- batch 1
            emit_attn(1, 2)
            emit_proj(7)
            emit_attn(1, 3)
            rD = emit_half_a2a(1, 1)          # cc3, trigger ~125us
            # recv chains emit AFTER the a2a staging so their cc-gated
            # gather DMAs can never delay staging on the in-order Sync queue
            args01 = emit_half_recv(0, 1, rB)  # cc1 done ~105us
            emit_attn(1, 0)
            # Wo(b0,h1) one section later than strictly needed: hardens
            # against long-barrier reps where cc1 completes ~115us
            emit_half_wo(*args01)             # PE ~140us
            emit_attn(1, 1)
            rC = emit_half_a2a(1, 0)          # cc4 (tail), trigger ~142us
            # tail: independent Wo blocks keep the PE busy through the
            # final collective + its recv chain
            args00 = emit_half_recv(0, 0, rA)  # cc2 done ~110us
            emit_half_wo(*args00)
            args11 = emit_half_recv(1, 1, rD, eng=nc.scalar)  # cc3 ~137us
            emit_half_wo(*args11)
            args10 = emit_half_recv(1, 0, rC, eng=nc.scalar)
            emit_half_wo(*args10)

    nc.compile()
    return nc


_NC = None


def _get_program():
    global _NC
    if _NC is None:
        _NC = build_program()
    return _NC


def _bf(a):
    return np.ascontiguousarray(a).astype(ml_dtypes.bfloat16)


def _f8(a):
    return np.ascontiguousarray(a).astype(ml_dtypes.float8_e4m3)


def kernel(x, Wq, bq, Wk, bk, Wv, bv, Wo, bo, _trace=False, _trace_kwargs=None):
    x = np.asarray(x, np.float32)
    Wq, Wk, Wv, Wo = (np.asarray(w, np.float32) for w in (Wq, Wk, Wv, Wo))
    bq, bk, bv, bo = (np.asarray(v, np.float32) for v in (bq, bk, bv, bo))

    xf = x.reshape(T, E).T
    xT = _bf(xf)
    x8T = _f8(xf)
    i = np.arange(128)
    tri = _bf((i[:, None] <= i[None, :]).astype(np.float32))
    ident = _bf(np.eye(128, dtype=np.float32))
    k16 = np.arange(16)[:, None]
    kcp = (2 * (np.arange(NKC * 128) // 128) + (np.arange(NKC * 128) % 128) // 64)
    selk = _bf((k16 == kcp[None, :]).astype(np.float32))

    in_maps = []
    for c in range(N_CORES):
        sl = slice(c * DPC, (c + 1) * DPC)
        in_maps.append({
            "xT": xT,
            "x8T": x8T,
            "wq8T": _f8(64.0 * Wq[sl, :].T),
            "wk8T": _f8(64.0 * Wk[sl, :].T),
            "wvT": _bf(Wv[sl, :].T),
            "woT": _bf(Wo.T),
            "bq": 64.0 * bq[sl].reshape(DPC, 1),
            "bk": 64.0 * bk[sl].reshape(DPC, 1),
            "bv": bv[sl].reshape(DPC, 1).copy(),
            "bo": bo,
            "tri": tri,
            "ident": ident,
            "selk": selk,
        })

    nc = _get_program()
    res = run_bass_kernel_spmd(nc, in_maps, list(range(N_CORES)),
                               trace=_trace, **(_trace_kwargs or {}))
    # out[c] rows are [batch, half, 128]: row (b, hf, r) holds global
    # token b*2048 + hf*1024 + c*128 + r.
    stacked = np.stack([res.results[i]["out"].reshape(B, 2, PH, E)
                        for i in range(N_CORES)], axis=2)
    full = stacked.reshape(T, E)
    if _trace:
        return full.reshape(B, S, E), res
    return full.reshape(B, S, E)
